# revision 1
# baseline (speedup 1.0000x reference)
"""MHA kernel for Trainium2, 8 NeuronCores.

Sharding: core c -> batch b = c//2, head-block hb = c%2 (8 of 16 heads).
Tensor-parallel within a batch: Wq/Wk/Wv column-sliced, Wo row-sliced;
each core emits a partial output [2048, 1024]; host sums the two partials
per batch and adds the bias (the "all-reduce" of row-parallel Wo done at
unshard time).

Per-core dataflow (all matmuls float32r = tf32-class, fp32 accumulate):
  xT   = PE-transpose(x)                     [dim,tok] 8x[128,2048]
  QT/KT[p] = (x @ W)^T via lhsT=W, rhs=xT    4x[128,2048] (pair p = 2 heads)
  V'   = x @ Wv with ones column per head    -> DRAM scratch [128,520]x16
  S^T duo: row-split K=64 pair (tile_position (0,0)/(64,0)) -> [128,1024] psum
  P^T  = exp(0.125 * S^T) on ACT             -> sbuf f32r
  O^T  = V'_h.T @ P^T_h (M=65, row 64 = softmax denominator)
  norm = recip(den) bcast via K=1 matmul, DVE multiply -> OT[p]
  out  = OT.T @ Wo (partial)                 [2048,1024] f32
"""
import numpy as np

import concourse.bacc as bacc
import concourse.mybir as mybir
from concourse.tile import TileContext
from concourse.bass_utils import run_bass_kernel_spmd

F32 = mybir.dt.float32
F32R = mybir.dt.float32r
AF = mybir.ActivationFunctionType

N = 2048      # tokens per batch
DIM = 1024    # model dim
HL = 512      # local inner (8 heads x 64)
NP = 4        # local head pairs
NJ = N // 128  # kv tiles
NQT = N // 512  # q tiles of 512
NK = DIM // 128  # contraction tiles

_CACHE = {}


def build():
    nc = bacc.Bacc(None, target_bir_lowering=False)
    x_d = nc.declare_dram_parameter("x", [N, DIM], F32R, isOutput=False)
    wq_d = nc.declare_dram_parameter("wq", [DIM, HL], F32R, isOutput=False)
    wk_d = nc.declare_dram_parameter("wk", [DIM, HL], F32R, isOutput=False)
    wv_d = nc.declare_dram_parameter("wv", [DIM, HL], F32R, isOutput=False)
    wo_d = nc.declare_dram_parameter("wo", [HL, DIM], F32R, isOutput=False)
    ones_d = nc.declare_dram_parameter("ones", [128, 64], F32R, isOutput=False)
    ident_d = nc.declare_dram_parameter("ident", [128, 128], F32R, isOutput=False)
    out_d = nc.declare_dram_parameter("out", [N, DIM], F32, isOutput=True)

    with TileContext(nc) as tc:
        with (
            tc.tile_pool(name="big", bufs=8) as big,      # xT then OT+Wo slots
            tc.tile_pool(name="qt", bufs=4) as qtp,
            tc.tile_pool(name="kt", bufs=4) as ktp,
            tc.tile_pool(name="w", bufs=9) as wp,
            tc.tile_pool(name="wsm", bufs=32) as wsm,
            tc.tile_pool(name="xin", bufs=2) as xinp,
            tc.tile_pool(name="pt", bufs=3) as ptp,
            tc.tile_pool(name="vp", bufs=18) as vpp,
            tc.tile_pool(name="st", bufs=3) as stp,
            tc.tile_pool(name="cn", bufs=1) as cn,
            tc.tile_pool(name="sps", bufs=2, space="PSUM") as spsp,
            tc.tile_pool(name="acc", bufs=1, space="PSUM") as accp,
            tc.tile_pool(name="dram", bufs=16, space="DRAM") as drp,
        ):
            ident = cn.tile([128, 128], F32R, name="ident")
            nc.sync.dma_start(out=ident[:], in_=ident_d[:])
            ones_sb = cn.tile([128, 64], F32R, name="ones_sb")
            nc.sync.dma_start(out=ones_sb[:], in_=ones_d[:])

            # ---- phase 1: transpose x -> xT[k] [128, N]
            xT = []
            for k in range(NK):
                t = big.tile([128, N], F32R, name=f"xT{k}", tag="big")
                xT.append(t)
            for tt in range(N // 128):
                xin = xinp.tile([128, DIM], F32R, name="xin")
                nc.sync.dma_start(out=xin[:], in_=x_d[tt * 128:(tt + 1) * 128, :])
                for k in range(NK):
                    tp = spsp.tile([128, 128], F32R, name="tp", tag="s")
                    nc.tensor.transpose(tp[:], xin[:, k * 128:(k + 1) * 128], ident[:])
                    nc.vector.tensor_copy(
                        out=xT[k][:, tt * 128:(tt + 1) * 128], in_=tp[:])

            # ---- phase 2+3 interleaved: V proj first, then per-pair
            # QT/KT projection + attention so proj PE hides under exp ACT.
            wv = []
            for k in range(NK):
                w = wp.tile([128, HL], F32R, name=f"wv{k}", tag="w")
                nc.sync.dma_start(out=w[:], in_=wv_d[k * 128:(k + 1) * 128, :])
                wv.append(w)
            vdr = []
            for tt in range(NJ):
                ps = spsp.tile([128, HL], F32, name="vps", tag="s")
                for k in range(NK):
                    nc.tensor.matmul(
                        ps[:], xT[k][:, tt * 128:(tt + 1) * 128], wv[k][:],
                        start=(k == 0), stop=(k == NK - 1))
                vst = ptp.tile([128, 520], F32R, name="vst", tag="pt")
                nc.vector.tensor_copy(out=vst[:, 64:520:65], in_=ones_sb[:, 0:8])
                for h in range(8):
                    nc.vector.tensor_copy(
                        out=vst[:, h * 65:h * 65 + 64],
                        in_=ps[:, h * 64:(h + 1) * 64])
                vd = drp.tile([128, 520], F32R, name=f"vd{tt}", tag="vd")
                nc.sync.dma_start(out=vd[:], in_=vst[:])
                vdr.append(vd)

            # Wo loads reuse freed xT slots
            wo = []
            for p in range(NP):
                w = big.tile([128, DIM], F32R, name=f"wo{p}", tag="big")
                nc.sync.dma_start(out=w[:], in_=wo_d[p * 128:(p + 1) * 128, :])
                wo.append(w)

            OT = [None] * NP
            for p in range(NP):
                # per-pair weight column slices [128, 128] x 8k for q and k
                wqp, wkp = [], []
                for k in range(NK):
                    w = wsm.tile([128, 128], F32R, name=f"wq{p}_{k}", tag="ws")
                    nc.sync.dma_start(
                        out=w[:], in_=wq_d[k * 128:(k + 1) * 128,
                                           p * 128:(p + 1) * 128])
                    wqp.append(w)
                    w = wsm.tile([128, 128], F32R, name=f"wk{p}_{k}", tag="ws")
                    nc.sync.dma_start(
                        out=w[:], in_=wk_d[k * 128:(k + 1) * 128,
                                           p * 128:(p + 1) * 128])
                    wkp.append(w)
                QTp = qtp.tile([128, N], F32R, name=f"QT{p}", tag="qt")
                KTp = ktp.tile([128, N], F32R, name=f"KT{p}", tag="kt")
                for t4 in range(NQT):
                    ps = spsp.tile([128, 512], F32, name="pps", tag="s")
                    for k in range(NK):
                        nc.tensor.matmul(
                            ps[:], wqp[k][:],
                            xT[k][:, t4 * 512:(t4 + 1) * 512],
                            start=(k == 0), stop=(k == NK - 1))
                    nc.vector.tensor_copy(
                        out=QTp[:, t4 * 512:(t4 + 1) * 512], in_=ps[:])
                    ps = spsp.tile([128, 512], F32, name="kps", tag="s")
                    for k in range(NK):
                        nc.tensor.matmul(
                            ps[:], wkp[k][:],
                            xT[k][:, t4 * 512:(t4 + 1) * 512],
                            start=(k == 0), stop=(k == NK - 1))
                    nc.vector.tensor_copy(
                        out=KTp[:, t4 * 512:(t4 + 1) * 512], in_=ps[:])

                pool_p = qtp if p % 2 == 0 else ktp
                OT[p] = pool_p.tile([128, N], F32R, name=f"OT{p}",
                                    tag="qt" if p % 2 == 0 else "kt")
                vtiles = []
                for j in range(NJ):
                    vj = vpp.tile([128, 130], F32R, name="vj", tag="vp")
                    nc.sync.dma_start(
                        out=vj[:], in_=vdr[j][:, p * 130:(p + 1) * 130])
                    vtiles.append(vj)
                for qt in range(NQT):
                    o_ps = [accp.tile([65, 512], F32, name=f"o{h}", tag=f"o{h}")
                            for h in range(2)]

                    def s_duo(j):
                        s_ps = spsp.tile([128, 1024], F32, name="s_ps", tag="s")
                        nc.tensor.matmul(
                            s_ps[:, 0:512], KTp[0:64, j * 128:(j + 1) * 128],
                            QTp[0:64, qt * 512:(qt + 1) * 512],
                            start=True, stop=True, tile_position=(0, 0))
                        nc.tensor.matmul(
                            s_ps[:, 512:1024], KTp[64:128, j * 128:(j + 1) * 128],
                            QTp[64:128, qt * 512:(qt + 1) * 512],
                            start=True, stop=True, tile_position=(64, 0))
                        return s_ps

                    # software pipeline: emit S(j+1) before attnV(j) so the
                    # in-order PE queue never stalls behind exp(j) on ACT.
                    s_cur = s_duo(0)
                    for j in range(NJ):
                        pt = ptp.tile([128, 1024], F32R, name="pt", tag="pt")
                        nc.scalar.activation(pt[:], s_cur[:], AF.Exp, scale=0.125)
                        if j + 1 < NJ:
                            s_cur = s_duo(j + 1)
                        for h in range(2):
                            nc.tensor.matmul(
                                o_ps[h][:], vtiles[j][:, h * 65:(h + 1) * 65],
                                pt[:, h * 512:(h + 1) * 512],
                                start=(j == 0), stop=(j == NJ - 1))
                    for h in range(2):
                        den = cn.tile([1, 512], F32R, name="den", tag="den",
                                      bufs=2)
                        with nc.allow_low_precision(reason="f32r==fp32 bits"):
                            nc.vector.reciprocal(den[0:1, :], o_ps[h][64:65, :])
                        bc_ps = accp.tile([64, 512], F32, name="bc", tag="bc",
                                          bufs=2)
                        nc.tensor.matmul(bc_ps[:], ones_sb[0:1, :], den[0:1, :],
                                         start=True, stop=True)
                        bc_sb = stp.tile([64, 512], F32, name="bc_sb", tag="st")
                        nc.vector.tensor_copy(out=bc_sb[:], in_=bc_ps[:])
                        nc.vector.tensor_tensor(
                            out=OT[p][h * 64:(h + 1) * 64,
                                      qt * 512:(qt + 1) * 512],
                            in0=o_ps[h][0:64, :], in1=bc_sb[:],
                            op=mybir.AluOpType.mult)

            # ---- phase 4: output projection (partial)
            for qs in range(N // 128):
                for dm in range(2):
                    ps = spsp.tile([128, 512], F32, name="ops", tag="s")
                    for p in range(NP):
                        nc.tensor.matmul(
                            ps[:], OT[p][:, qs * 128:(qs + 1) * 128],
                            wo[p][:, dm * 512:(dm + 1) * 512],
                            start=(p == 0), stop=(p == NP - 1))
                    ost = stp.tile([128, 512], F32, name="ost", tag="st")
                    nc.vector.tensor_copy(out=ost[:], in_=ps[:])
                    nc.sync.dma_start(
                        out=out_d[qs * 128:(qs + 1) * 128,
                                  dm * 512:(dm + 1) * 512],
                        in_=ost[:])
    nc.finalize()
    return nc


def kernel(x, Wq, Wk, Wv, Wo, bo, _trace=False):
    x = np.asarray(x, np.float32)
    Wq, Wk, Wv, Wo = (np.asarray(a, np.float32) for a in (Wq, Wk, Wv, Wo))
    bo = np.asarray(bo, np.float32)

    if "nc" not in _CACHE:
        _CACHE["nc"] = build()
    nc = _CACHE["nc"]

    ones_in = np.ones((128, 64), np.float32)
    ident_in = np.eye(128, dtype=np.float32)
    in_maps = []
    for c in range(8):
        b, hb = c // 2, c % 2
        sl = slice(hb * 512, (hb + 1) * 512)
        in_maps.append({
            "x": np.ascontiguousarray(x[b]),
            "wq": np.ascontiguousarray(Wq[:, sl]),
            "wk": np.ascontiguousarray(Wk[:, sl]),
            "wv": np.ascontiguousarray(Wv[:, sl]),
            "wo": np.ascontiguousarray(Wo[sl, :]),
            "ones": ones_in, "ident": ident_in,
        })
    res = run_bass_kernel_spmd(nc, in_maps, list(range(8)), trace=_trace)
    out = np.empty((4, N, DIM), np.float32)
    for b in range(4):
        out[b] = res.results[2 * b]["out"] + res.results[2 * b + 1]["out"] + bo
    if _trace:
        return out, res
    return out



# revision 30
# speedup vs baseline: 1.3429x; 1.3429x over previous
"""MHA kernel for Trainium2, 8 NeuronCores.

Sharding: core c -> batch b = c//2, head-block hb = c%2 (8 of 16 heads).
Tensor-parallel within a batch: Wq/Wk/Wv column-sliced, Wo row-sliced;
each core emits a partial output [2048, 1024]; host sums the two partials
per batch and adds the bias.

All PE operands are bf16 (psum accumulation stays fp32); the host packs
x^T and the weights into partition-major blocks loaded in a few large DMAs.

Per-core dataflow (flash-attention style, q-block outer):
  QT/KT[p] = (Wq/Wk pair-slice)^T-proj          4x[128, 2048] bf16 sbuf
  V'[j]    = [x @ Wv | 1] per head              16x[128, 8x65] bf16 sbuf
  loop qb(2 q-blocks of 1024) x h(8) x j(16 kv-tiles):
    S^T  = KT_h[:,j].T @ QT_h[:,qb]  psum [128kv, 1024q]   (2 mm, K=64)
    P^T  = exp(0.125 S^T) on ACT  -> bf16 sbuf (persists one full head)
  attnV for head h runs as *backlog* during head h+1's stream: per q-tile
  qtl, one sequential psum accumulation group (its own bank, legal on HW):
    O[q, 65] = sum_j P^T_hj[:,qtl].T @ V'_j[:,h]   (16 mm, N=65;
               column 64 accumulates the softmax denominator)
  then per qtl: recip(den) + O*recip -> ONP bf16 (qt-major pair layout);
  per pair: PE-transpose ONP -> OT[p]; per qb: out[qt] = sum_p OT_p.T@Wo_p.
QKV projections and q-block 0's output projection are interleaved into the
main loop as background PE work (deadline-scheduled) so the PE never idles.
"""
import numpy as np
from collections import deque

import concourse.bacc as bacc
import concourse.bass as bass
import concourse.mybir as mybir
from concourse.tile import TileContext
from concourse.bass_utils import run_bass_kernel_spmd

F32 = mybir.dt.float32
BF16 = mybir.dt.bfloat16
AF = mybir.ActivationFunctionType

N = 2048       # tokens per batch
DIM = 1024     # model dim
HL = 512       # local inner (8 heads x 64)
NP = 4         # local head pairs
NJ = 16        # kv tiles of 128
NQB = 2        # q blocks of 1024
NQT = 8        # q tiles of 128 per q block
NK = 8         # contraction tiles of 128 over DIM

_CACHE = {}


def build():
    nc = bacc.Bacc(None, target_bir_lowering=False)
    # packed inputs: partition-major blocks.  x^T is packed as two column
    # halves (k-major within each) so the prelude only waits for the first.
    xt_d = nc.declare_dram_parameter("xtp", [128, NK * N], BF16, isOutput=False)
    wqk_d = nc.declare_dram_parameter("wqk", [128, NK * 2 * HL], BF16,
                                      isOutput=False)
    wv_d = nc.declare_dram_parameter("wvp", [128, NK * HL], BF16,
                                     isOutput=False)
    wo_d = nc.declare_dram_parameter("wop", [128, NP * DIM], BF16,
                                     isOutput=False)
    idon_d = nc.declare_dram_parameter("idon", [128, 129], BF16, isOutput=False)
    out_d = nc.declare_dram_parameter("out", [N, DIM], BF16, isOutput=True)

    with TileContext(nc) as tc:
        with (
            tc.tile_pool(name="cn", bufs=1) as cn,       # constants + weights
            tc.tile_pool(name="act", bufs=1) as actp,    # QT/KT/V activations
            tc.tile_pool(name="pt", bufs=22) as ptp,     # P^T exp outputs
            tc.tile_pool(name="on", bufs=2) as onp_p,    # normalized O + O^T
            tc.tile_pool(name="sm", bufs=2) as smp,      # small: denr, out_sb
            tc.tile_pool(name="ps", bufs=1, space="PSUM") as psp,
        ):
            # ---- packed input DMAs, all on the SP queue in priority order
            idon = cn.tile([128, 129], BF16, name="idon", tag="idon")
            nc.sync.dma_start(out=idon[:], in_=idon_d[:])
            ident = idon[:, 0:128]
            wqks = cn.tile([128, NK * 2 * HL], BF16, name="wqks", tag="wqks")
            for c in range(2):
                nc.sync.dma_start(out=wqks[:, c * 4096:(c + 1) * 4096],
                                  in_=wqk_d[:, c * 4096:(c + 1) * 4096])
            xtsA = cn.tile([128, NK * 1024], BF16, name="xtsA", tag="xtsA")
            xtsB = cn.tile([128, NK * 1024], BF16, name="xtsB", tag="xtsB")
            for c in range(2):
                nc.sync.dma_start(out=xtsA[:, c * 4096:(c + 1) * 4096],
                                  in_=xt_d[:, c * 4096:(c + 1) * 4096])
            wvs = cn.tile([128, NK * HL], BF16, name="wvs", tag="wvs")
            nc.sync.dma_start(out=wvs[:], in_=wv_d[:])
            for c in range(2):
                nc.sync.dma_start(out=xtsB[:, c * 4096:(c + 1) * 4096],
                                  in_=xt_d[:, 8192 + c * 4096:8192 + (c + 1) * 4096])
            wos = cn.tile([128, NP * DIM], BF16, name="wos", tag="wos")
            nc.sync.dma_start(out=wos[:], in_=wo_d[:])

            def xt_cols(k, c0, w):
                # slice [c0, c0+w) of logical x^T k-chunk; never straddles
                # the 1024-column A/B boundary by construction
                if c0 < 1024:
                    assert c0 + w <= 1024
                    return xtsA[:, k * 1024 + c0:k * 1024 + c0 + w]
                return xtsB[:, k * 1024 + c0 - 1024:k * 1024 + c0 - 1024 + w]

            def wq(k):
                return wqks[:, k * 1024:k * 1024 + 512]

            def wk(k):
                return wqks[:, k * 1024 + 512:k * 1024 + 1024]

            def wv(k):
                return wvs[:, k * HL:(k + 1) * HL]

            def wo(p):
                return wos[:, p * DIM:(p + 1) * DIM]

            # ---- persistent activation tiles
            qt = [actp.tile([128, N], BF16, name=f"qt{p}", tag=f"qt{p}")
                  for p in range(NP)]
            kt = [actp.tile([128, N], BF16, name=f"kt{p}", tag=f"kt{p}")
                  for p in range(NP)]
            # V' = [V_h | 1] per head: 65 columns per head
            v = [actp.tile([128, 8 * 65], BF16, name=f"v{j}", tag=f"v{j}")
                 for j in range(NJ)]

            # ---- PE warmup: dummy matmuls on a memset tile keep the
            # p-state ramp busy while the input DMAs stream in.
            wsrc = cn.tile([128, 512], BF16, name="wsrc", tag="wsrc")
            nc.gpsimd.memset(wsrc[:], 0.0)
            # fill every V' tile with 1.0 once; the per-head 64-column
            # blocks are overwritten by the V projection copies, leaving
            # the denominator ones-columns (64::65) intact
            for j in range(NJ):
                nc.vector.memset(v[j][:], 1.0)
            warm = psp.tile([128, 512], F32, name="warm", tag="pj", bufs=2)

            def dummies(n):
                for _ in range(n):
                    nc.tensor.matmul(warm[:], wsrc[:, 0:128], wsrc[:],
                                     start=True, stop=True)

            # ---- background PE work: projection groups through the 2-bank
            # "pj" psum slots
            def emit_qkt_group(which, p, ncc, tag="pj"):
                wf = wq if which == "q" else wk
                dst = qt[p] if which == "q" else kt[p]
                ps = psp.tile([128, 512], F32, name="pj", tag=tag, bufs=2)
                for k in range(NK):
                    nc.tensor.matmul(
                        ps[:], wf(k)[:, p * 128:(p + 1) * 128],
                        xt_cols(k, ncc * 512, 512),
                        start=(k == 0), stop=(k == NK - 1))
                nc.vector.tensor_copy(
                    out=dst[:, ncc * 512:(ncc + 1) * 512], in_=ps[:])

            def emit_v_group(j, tag="pj"):
                ps = psp.tile([128, 512], F32, name="pj", tag=tag, bufs=2)
                for k in range(NK):
                    nc.tensor.matmul(
                        ps[:], xt_cols(k, j * 128, 128), wv(k),
                        start=(k == 0), stop=(k == NK - 1))
                # per-head rectangular copies into the 65-col V' layout
                for hh_ in range(8):
                    nc.vector.tensor_copy(
                        out=v[j][:, hh_ * 65:hh_ * 65 + 64],
                        in_=ps[:, hh_ * 64:(hh_ + 1) * 64])

            def emit_outproj_qtl(qb, qtl, tail=False):
                ots = [ot_tiles[(qb, p)] for p in range(NP)]
                r0 = qb * 1024 + qtl * 128
                osb = smp.tile([128, DIM], BF16, name="osb", tag="osb", bufs=4)
                if tail:
                    # s-tag psum banks are free after the last exp; the two
                    # dm-halves are sequential groups per bank (legal).
                    ps = psp.tile([128, 1024], F32, name="s", tag="s", bufs=2)
                    for dm in range(2):
                        for p in range(NP):
                            nc.tensor.matmul(
                                ps[:, dm * 512:(dm + 1) * 512],
                                ots[p][:, qtl * 128:(qtl + 1) * 128],
                                wo(p)[:, dm * 512:(dm + 1) * 512],
                                start=(p == 0), stop=(p == NP - 1))
                    if qtl % 2 == 0:
                        nc.scalar.copy(out=osb[:], in_=ps[:])
                    else:
                        nc.vector.tensor_copy(out=osb[:], in_=ps[:])
                else:
                    for dm in range(2):
                        ps = psp.tile([128, 512], F32, name="pj", tag="pj",
                                      bufs=2)
                        for p in range(NP):
                            nc.tensor.matmul(
                                ps[:], ots[p][:, qtl * 128:(qtl + 1) * 128],
                                wo(p)[:, dm * 512:(dm + 1) * 512],
                                start=(p == 0), stop=(p == NP - 1))
                        nc.vector.tensor_copy(
                            out=osb[:, dm * 512:(dm + 1) * 512], in_=ps[:])
                nc.sync.dma_start(out=out_d[r0:r0 + 128, :], in_=osb[:])

            # ---- attnV backlog: head h's attention-times-V runs during
            # head h+1's S/exp stream, one legal psum group per q-tile.
            ot_tiles = {}
            onp_tiles = {}

            def emit_attnv_qtl(pts, h, qb, qtl, slot):
                o_t = psp.tile([128, 65], F32, name="oq", tag=f"o{slot}",
                               bufs=1)
                for j in range(NJ):
                    nc.tensor.matmul(
                        o_t[:],
                        pts[j][:, qtl * 128:(qtl + 1) * 128],
                        v[j][:, h * 65:h * 65 + 65],
                        start=(j == 0), stop=(j == NJ - 1))
                # normalize: ONP[:, qtl*128 + hh*64 + d] = O[:, d] / O[:, 64]
                p, hh = h // 2, h % 2
                if (p, qb) not in onp_tiles:
                    onp_tiles[(p, qb)] = onp_p.tile(
                        [128, 1024], BF16, name=f"onp{p}", tag=f"onp{p}")
                onp_t = onp_tiles[(p, qb)]
                denr = smp.tile([128, 1], F32, name="denr", tag="denr", bufs=4)
                with nc.allow_low_precision(reason="fp32 recip"):
                    nc.vector.reciprocal(denr[:], o_t[:, 64:65])
                nc.vector.tensor_scalar_mul(
                    onp_t[:, qtl * 128 + hh * 64:qtl * 128 + hh * 64 + 64],
                    o_t[:, 0:64], denr[:, 0:1])

            def finish_pair(p, qb):
                # PE-transpose ONP -> OT_p [128 d, 1024 q] for this q block
                onp_t = onp_tiles[(p, qb)]
                tp = psp.tile([128, 1024], BF16, name="tp", tag="pj", bufs=2)
                for qtl in range(NQT):
                    nc.tensor.transpose(
                        tp[:, qtl * 128:(qtl + 1) * 128],
                        onp_t[:, qtl * 128:(qtl + 1) * 128], ident)
                ot = onp_p.tile([128, 1024], BF16, name=f"ot{p}", tag=f"ot{p}")
                nc.vector.tensor_copy(out=ot[:], in_=tp[:])
                ot_tiles[(qb, p)] = ot

            # background emission schedule: iter t = (qb*8 + h)*16 + j.
            # (deadline_iter, earliest_iter, fn, args)
            groups = []
            for j in range(1, NJ):
                # all V' tiles are read by head-0's backlog from iter 16
                groups.append((j, 0, emit_v_group, (j,)))
            for p in range(NP):
                for c in range(4):  # KT chunk c first read at t=32p+4c
                    if p == 0 and c == 0:
                        continue  # prelude
                    groups.append((32 * p + 4 * c - 3, 0,
                                   emit_qkt_group, ("k", p, c)))
                for ncc in range(4):
                    if p == 0 and ncc in (0, 1):
                        continue  # prelude
                    first = 128 * (ncc // 2) + 32 * p
                    groups.append((first - 4, 0, emit_qkt_group, ("q", p, ncc)))
            for qtl in range(NQT):  # q-block 0 outproj during q-block 1
                groups.append((250, 140 + 2 * qtl, emit_outproj_qtl, (0, qtl)))

            q_bg = deque(sorted(groups, key=lambda g: (g[0], g[1])))
            bg = {}
            last_emit = -10
            for tt in range(NQB * 8 * NJ):
                while q_bg and q_bg[0][0] <= tt:
                    g = q_bg.popleft()
                    bg.setdefault(tt, []).append((g[2], g[3]))
                    last_emit = tt
                if q_bg and tt - last_emit >= 2 and q_bg[0][1] <= tt:
                    g = q_bg.popleft()
                    bg.setdefault(tt, []).append((g[2], g[3]))
                    last_emit = tt

            # ---- prelude: just enough for S(h0, j0)
            dummies(14)
            emit_qkt_group("q", 0, 0, tag="s")
            dummies(4)
            emit_qkt_group("q", 0, 1, tag="s")
            dummies(4)
            emit_qkt_group("k", 0, 0, tag="pj")
            emit_v_group(0)

            # ---- main loop
            cur_pts = []          # P^T tiles of the in-flight head
            backlog = deque()     # (pts, h, qb, qtl) attnV tasks
            slot_ctr = 0
            pend_pair = None

            for t in range(NQB * 8 * NJ):
                qb, r = divmod(t, 8 * NJ)
                h, i = divmod(r, NJ)
                hh, p = h % 2, h // 2
                j = i
                # S^T tile for (qb, h, j): two bank-sized single groups
                s_ps = psp.tile([128, 1024], F32, name="s", tag="s", bufs=2)
                for sc in range(2):
                    nc.tensor.matmul(
                        s_ps[:, sc * 512:(sc + 1) * 512],
                        kt[p][hh * 64:hh * 64 + 64, j * 128:(j + 1) * 128],
                        qt[p][hh * 64:hh * 64 + 64,
                              qb * 1024 + sc * 512:qb * 1024 + (sc + 1) * 512],
                        start=True, stop=True)
                pt_t = ptp.tile([128, 1024], BF16, name="pt", tag="pt")
                nc.scalar.activation(pt_t[:], s_ps[:], AF.Exp, scale=0.125)
                cur_pts.append(pt_t)
                # drain up to 3 backlog attnV groups (prev head's); drain
                # BEFORE the fill so a head's groups start strictly after
                # its last iteration (all V'/P^T writers already emitted)
                for _ in range(3):
                    if not backlog:
                        break
                    pts_, bh, bqb, bqtl = backlog.popleft()
                    emit_attnv_qtl(pts_, bh, bqb, bqtl, slot_ctr % 2)
                    slot_ctr += 1
                    if bqtl == NQT - 1 and bh % 2 == 1:
                        pend_pair = (bh // 2, bqb)
                if pend_pair is not None and not backlog:
                    finish_pair(*pend_pair)
                    pend_pair = None
                if i == NJ - 1:
                    # head finished streaming: queue its attnV backlog
                    for qtl in range(NQT):
                        backlog.append((cur_pts, h, qb, qtl))
                    cur_pts = []
                # background projection / output-projection groups
                for fn, a in bg.get(t, ()):
                    fn(*a)

            # drain: last head's backlog, its pair, final outproj
            while backlog:
                pts_, bh, bqb, bqtl = backlog.popleft()
                emit_attnv_qtl(pts_, bh, bqb, bqtl, slot_ctr % 2)
                slot_ctr += 1
            finish_pair(3, NQB - 1)
            for qtl in range(NQT):
                emit_outproj_qtl(NQB - 1, qtl, tail=True)
    nc.finalize()
    return nc


def kernel(x, Wq, Wk, Wv, Wo, bo, _trace=False):
    npdt = mybir.dt.np(BF16)
    x = np.asarray(x, np.float32)
    bo = np.asarray(bo, np.float32)
    Wq, Wk, Wv = (np.asarray(a, np.float32) for a in (Wq, Wk, Wv))
    Wo = np.asarray(Wo, np.float32)

    if "nc" not in _CACHE:
        _CACHE["nc"] = build()
    nc = _CACHE["nc"]

    idon_in = np.concatenate(
        [np.eye(128, dtype=np.float32),
         np.ones((128, 1), np.float32)], axis=1).astype(npdt)
    in_maps = []
    for c in range(8):
        b, hb = c // 2, c % 2
        sl = slice(hb * HL, (hb + 1) * HL)
        xT = x[b].T  # [1024, 2048]
        xA = xT[:, 0:1024].reshape(NK, 128, 1024).transpose(1, 0, 2)
        xB = xT[:, 1024:2048].reshape(NK, 128, 1024).transpose(1, 0, 2)
        xtp = np.concatenate([xA.reshape(128, NK * 1024),
                              xB.reshape(128, NK * 1024)], axis=1)
        wqk = np.concatenate(
            [Wq[:, sl].reshape(NK, 128, HL),
             Wk[:, sl].reshape(NK, 128, HL)],
            axis=2).transpose(1, 0, 2).reshape(128, NK * 2 * HL)
        wvp = Wv[:, sl].reshape(NK, 128, HL).transpose(1, 0, 2).reshape(
            128, NK * HL)
        wop = Wo[sl, :].reshape(NP, 128, DIM).transpose(1, 0, 2).reshape(
            128, NP * DIM)
        in_maps.append({
            "xtp": np.ascontiguousarray(xtp).astype(npdt),
            "wqk": np.ascontiguousarray(wqk).astype(npdt),
            "wvp": np.ascontiguousarray(wvp).astype(npdt),
            "wop": np.ascontiguousarray(wop).astype(npdt),
            "idon": idon_in,
        })
    res = run_bass_kernel_spmd(nc, in_maps, list(range(8)), trace=_trace)
    out = np.empty((4, N, DIM), np.float32)
    for b in range(4):
        out[b] = (res.results[2 * b]["out"].astype(np.float32)
                  + res.results[2 * b + 1]["out"].astype(np.float32) + bo)
    if _trace:
        return out, res
    return out


# revision 31
# speedup vs baseline: 1.3499x; 1.0052x over previous
"""MHA kernel for Trainium2, 8 NeuronCores.

Sharding: core c -> batch b = c//2, head-block hb = c%2 (8 of 16 heads).
Tensor-parallel within a batch: Wq/Wk/Wv column-sliced, Wo row-sliced;
each core emits a partial output [2048, 1024]; host sums the two partials
per batch and adds the bias.

All PE operands are bf16 (psum accumulation stays fp32); the host packs
x^T and the weights into partition-major blocks loaded in a few large DMAs.

Per-core dataflow (flash-attention style, q-block outer):
  QT/KT[p] = (Wq/Wk pair-slice)^T-proj          4x[128, 2048] bf16 sbuf
  V'[j]    = [x @ Wv | 1] per head              16x[128, 8x65] bf16 sbuf
  loop qb(2 q-blocks of 1024) x h(8) x j(16 kv-tiles):
    S^T  = KT_h[:,j].T @ QT_h[:,qb]  psum [128kv, 1024q]   (2 mm, K=64)
    P^T  = exp(0.125 S^T) on ACT  -> bf16 sbuf (persists one full head)
  attnV for head h runs as *backlog* during head h+1's stream: per q-tile
  qtl, one sequential psum accumulation group (its own bank, legal on HW):
    O[q, 65] = sum_j P^T_hj[:,qtl].T @ V'_j[:,h]   (16 mm, N=65;
               column 64 accumulates the softmax denominator)
  then per qtl: recip(den) + O*recip -> ONP bf16 (qt-major pair layout);
  per pair: PE-transpose ONP -> OT[p]; per qb: out[qt] = sum_p OT_p.T@Wo_p.
QKV projections and q-block 0's output projection are interleaved into the
main loop as background PE work (deadline-scheduled) so the PE never idles.
"""
import numpy as np
from collections import deque

import concourse.bacc as bacc
import concourse.bass as bass
import concourse.mybir as mybir
from concourse.tile import TileContext
from concourse.bass_utils import run_bass_kernel_spmd

F32 = mybir.dt.float32
BF16 = mybir.dt.bfloat16
AF = mybir.ActivationFunctionType

N = 2048       # tokens per batch
DIM = 1024     # model dim
HL = 512       # local inner (8 heads x 64)
NP = 4         # local head pairs
NJ = 16        # kv tiles of 128
NQB = 2        # q blocks of 1024
NQT = 8        # q tiles of 128 per q block
NK = 8         # contraction tiles of 128 over DIM

_CACHE = {}


def build():
    nc = bacc.Bacc(None, target_bir_lowering=False)
    # packed inputs: partition-major blocks.  x^T is packed as two column
    # halves (k-major within each) so the prelude only waits for the first.
    xt_d = nc.declare_dram_parameter("xtp", [128, NK * N], BF16, isOutput=False)
    wqk_d = nc.declare_dram_parameter("wqk", [128, NK * 2 * HL], BF16,
                                      isOutput=False)
    wqk0_d = nc.declare_dram_parameter("wqk0", [128, NK * 256], BF16,
                                       isOutput=False)
    wv_d = nc.declare_dram_parameter("wvp", [128, NK * HL], BF16,
                                     isOutput=False)
    wo_d = nc.declare_dram_parameter("wop", [128, NP * DIM], BF16,
                                     isOutput=False)
    idon_d = nc.declare_dram_parameter("idon", [128, 129], BF16, isOutput=False)
    out_d = nc.declare_dram_parameter("out", [N, DIM], BF16, isOutput=True)

    with TileContext(nc) as tc:
        with (
            tc.tile_pool(name="cn", bufs=1) as cn,       # constants + weights
            tc.tile_pool(name="act", bufs=1) as actp,    # QT/KT/V activations
            tc.tile_pool(name="pt", bufs=22) as ptp,     # P^T exp outputs
            tc.tile_pool(name="on", bufs=2) as onp_p,    # normalized O + O^T
            tc.tile_pool(name="sm", bufs=2) as smp,      # small: denr, out_sb
            tc.tile_pool(name="ps", bufs=1, space="PSUM") as psp,
        ):
            # ---- packed input DMAs, all on the SP queue in priority order
            idon = cn.tile([128, 129], BF16, name="idon", tag="idon")
            nc.sync.dma_start(out=idon[:], in_=idon_d[:])
            ident = idon[:, 0:128]
            wqk0s = cn.tile([128, NK * 256], BF16, name="wqk0s", tag="wqk0s")
            nc.sync.dma_start(out=wqk0s[:], in_=wqk0_d[:])
            xtsA = cn.tile([128, NK * 1024], BF16, name="xtsA", tag="xtsA")
            xtsB = cn.tile([128, NK * 1024], BF16, name="xtsB", tag="xtsB")
            for c in range(4):
                nc.sync.dma_start(out=xtsA[:, c * 2048:(c + 1) * 2048],
                                  in_=xt_d[:, c * 2048:(c + 1) * 2048])
            wvs = cn.tile([128, NK * HL], BF16, name="wvs", tag="wvs")
            nc.sync.dma_start(out=wvs[:], in_=wv_d[:])
            wqks = cn.tile([128, NK * 2 * HL], BF16, name="wqks", tag="wqks")
            for c in range(2):
                nc.sync.dma_start(out=wqks[:, c * 4096:(c + 1) * 4096],
                                  in_=wqk_d[:, c * 4096:(c + 1) * 4096])
            for c in range(2):
                nc.sync.dma_start(out=xtsB[:, c * 4096:(c + 1) * 4096],
                                  in_=xt_d[:, 8192 + c * 4096:8192 + (c + 1) * 4096])
            wos = cn.tile([128, NP * DIM], BF16, name="wos", tag="wos")
            nc.sync.dma_start(out=wos[:], in_=wo_d[:])

            def xt_cols(k, c0, w):
                # slice [c0, c0+w) of logical x^T k-chunk; never straddles
                # the 1024-column A/B boundary by construction
                if c0 < 1024:
                    assert c0 + w <= 1024
                    return xtsA[:, k * 1024 + c0:k * 1024 + c0 + w]
                return xtsB[:, k * 1024 + c0 - 1024:k * 1024 + c0 - 1024 + w]

            def wq(k):
                return wqks[:, k * 1024:k * 1024 + 512]

            def wk(k):
                return wqks[:, k * 1024 + 512:k * 1024 + 1024]

            def wv(k):
                return wvs[:, k * HL:(k + 1) * HL]

            def wo(p):
                return wos[:, p * DIM:(p + 1) * DIM]

            # ---- persistent activation tiles
            qt = [actp.tile([128, N], BF16, name=f"qt{p}", tag=f"qt{p}")
                  for p in range(NP)]
            kt = [actp.tile([128, N], BF16, name=f"kt{p}", tag=f"kt{p}")
                  for p in range(NP)]
            # V' = [V_h | 1] per head: 65 columns per head
            v = [actp.tile([128, 8 * 65], BF16, name=f"v{j}", tag=f"v{j}")
                 for j in range(NJ)]

            # ---- PE warmup: dummy matmuls on a memset tile keep the
            # p-state ramp busy while the input DMAs stream in.
            wsrc = cn.tile([128, 512], BF16, name="wsrc", tag="wsrc")
            nc.gpsimd.memset(wsrc[:], 0.0)
            # fill every V' tile with 1.0 once; the per-head 64-column
            # blocks are overwritten by the V projection copies, leaving
            # the denominator ones-columns (64::65) intact
            for j in range(NJ):
                nc.vector.memset(v[j][:], 1.0)
            warm = psp.tile([128, 512], F32, name="warm", tag="pj", bufs=2)

            def dummies(n):
                for _ in range(n):
                    nc.tensor.matmul(warm[:], wsrc[:, 0:128], wsrc[:],
                                     start=True, stop=True)

            # ---- background PE work: projection groups through the 2-bank
            # "pj" psum slots
            def emit_qkt_group(which, p, ncc, tag="pj", pre=False):
                if pre:
                    wf = (lambda k: wqk0s[:, k * 256:k * 256 + 128]) \
                        if which == "q" else \
                        (lambda k: wqk0s[:, k * 256 + 128:k * 256 + 256])
                else:
                    wf = wq if which == "q" else wk
                dst = qt[p] if which == "q" else kt[p]
                ps = psp.tile([128, 512], F32, name="pj", tag=tag, bufs=2)
                for k in range(NK):
                    w_k = wf(k) if pre else wf(k)[:, p * 128:(p + 1) * 128]
                    nc.tensor.matmul(
                        ps[:], w_k, xt_cols(k, ncc * 512, 512),
                        start=(k == 0), stop=(k == NK - 1))
                nc.vector.tensor_copy(
                    out=dst[:, ncc * 512:(ncc + 1) * 512], in_=ps[:])

            def emit_v_group(j, tag="pj"):
                ps = psp.tile([128, 512], F32, name="pj", tag=tag, bufs=2)
                for k in range(NK):
                    nc.tensor.matmul(
                        ps[:], xt_cols(k, j * 128, 128), wv(k),
                        start=(k == 0), stop=(k == NK - 1))
                # per-head rectangular copies into the 65-col V' layout
                for hh_ in range(8):
                    nc.vector.tensor_copy(
                        out=v[j][:, hh_ * 65:hh_ * 65 + 64],
                        in_=ps[:, hh_ * 64:(hh_ + 1) * 64])

            def emit_outproj_qtl(qb, qtl, tail=False):
                ots = [ot_tiles[(qb, p)] for p in range(NP)]
                r0 = qb * 1024 + qtl * 128
                osb = smp.tile([128, DIM], BF16, name="osb", tag="osb", bufs=4)
                if tail:
                    # s-tag psum banks are free after the last exp; the two
                    # dm-halves are sequential groups per bank (legal).
                    ps = psp.tile([128, 1024], F32, name="s", tag="s", bufs=2)
                    for dm in range(2):
                        for p in range(NP):
                            nc.tensor.matmul(
                                ps[:, dm * 512:(dm + 1) * 512],
                                ots[p][:, qtl * 128:(qtl + 1) * 128],
                                wo(p)[:, dm * 512:(dm + 1) * 512],
                                start=(p == 0), stop=(p == NP - 1))
                    if qtl % 2 == 0:
                        nc.scalar.copy(out=osb[:], in_=ps[:])
                    else:
                        nc.vector.tensor_copy(out=osb[:], in_=ps[:])
                else:
                    for dm in range(2):
                        ps = psp.tile([128, 512], F32, name="pj", tag="pj",
                                      bufs=2)
                        for p in range(NP):
                            nc.tensor.matmul(
                                ps[:], ots[p][:, qtl * 128:(qtl + 1) * 128],
                                wo(p)[:, dm * 512:(dm + 1) * 512],
                                start=(p == 0), stop=(p == NP - 1))
                        nc.vector.tensor_copy(
                            out=osb[:, dm * 512:(dm + 1) * 512], in_=ps[:])
                nc.sync.dma_start(out=out_d[r0:r0 + 128, :], in_=osb[:])

            # ---- attnV backlog: head h's attention-times-V runs during
            # head h+1's S/exp stream, one legal psum group per q-tile.
            ot_tiles = {}
            onp_tiles = {}

            def emit_attnv_qtl(pts, h, qb, qtl, slot):
                o_t = psp.tile([128, 65], F32, name="oq", tag=f"o{slot}",
                               bufs=1)
                for j in range(NJ):
                    nc.tensor.matmul(
                        o_t[:],
                        pts[j][:, qtl * 128:(qtl + 1) * 128],
                        v[j][:, h * 65:h * 65 + 65],
                        start=(j == 0), stop=(j == NJ - 1))
                # normalize: ONP[:, qtl*128 + hh*64 + d] = O[:, d] / O[:, 64]
                p, hh = h // 2, h % 2
                if (p, qb) not in onp_tiles:
                    onp_tiles[(p, qb)] = onp_p.tile(
                        [128, 1024], BF16, name=f"onp{p}", tag=f"onp{p}")
                onp_t = onp_tiles[(p, qb)]
                denr = smp.tile([128, 1], F32, name="denr", tag="denr", bufs=4)
                with nc.allow_low_precision(reason="fp32 recip"):
                    nc.vector.reciprocal(denr[:], o_t[:, 64:65])
                nc.vector.tensor_scalar_mul(
                    onp_t[:, qtl * 128 + hh * 64:qtl * 128 + hh * 64 + 64],
                    o_t[:, 0:64], denr[:, 0:1])

            def finish_pair(p, qb):
                # PE-transpose ONP -> OT_p [128 d, 1024 q] for this q block
                onp_t = onp_tiles[(p, qb)]
                tp = psp.tile([128, 1024], BF16, name="tp", tag="pj", bufs=2)
                for qtl in range(NQT):
                    nc.tensor.transpose(
                        tp[:, qtl * 128:(qtl + 1) * 128],
                        onp_t[:, qtl * 128:(qtl + 1) * 128], ident)
                ot = onp_p.tile([128, 1024], BF16, name=f"ot{p}", tag=f"ot{p}")
                nc.vector.tensor_copy(out=ot[:], in_=tp[:])
                ot_tiles[(qb, p)] = ot

            # background emission schedule: iter t = (qb*8 + h)*16 + j.
            # (deadline_iter, earliest_iter, fn, args)
            groups = []
            for j in range(1, NJ):
                # all V' tiles are read by head-0's backlog from iter 16
                groups.append((j, 0, emit_v_group, (j,)))
            for p in range(NP):
                for c in range(4):  # KT chunk c first read at t=32p+4c
                    if p == 0 and c == 0:
                        continue  # prelude
                    groups.append((32 * p + 4 * c - 3, 0,
                                   emit_qkt_group, ("k", p, c)))
                for ncc in range(4):
                    if p == 0 and ncc in (0, 1):
                        continue  # prelude
                    first = 128 * (ncc // 2) + 32 * p
                    groups.append((first - 4, 0, emit_qkt_group, ("q", p, ncc)))
            for qtl in range(NQT):  # q-block 0 outproj during q-block 1
                groups.append((250, 140 + 2 * qtl, emit_outproj_qtl, (0, qtl)))

            q_bg = deque(sorted(groups, key=lambda g: (g[0], g[1])))
            bg = {}
            last_emit = -10
            for tt in range(NQB * 8 * NJ):
                while q_bg and q_bg[0][0] <= tt:
                    g = q_bg.popleft()
                    bg.setdefault(tt, []).append((g[2], g[3]))
                    last_emit = tt
                if q_bg and tt - last_emit >= 2 and q_bg[0][1] <= tt:
                    g = q_bg.popleft()
                    bg.setdefault(tt, []).append((g[2], g[3]))
                    last_emit = tt

            # ---- prelude: just enough for S(h0, j0)
            dummies(14)
            emit_qkt_group("q", 0, 0, tag="s", pre=True)
            dummies(4)
            emit_qkt_group("q", 0, 1, tag="s", pre=True)
            dummies(4)
            emit_qkt_group("k", 0, 0, tag="pj", pre=True)
            emit_v_group(0)

            # ---- main loop
            cur_pts = []          # P^T tiles of the in-flight head
            backlog = deque()     # (pts, h, qb, qtl) attnV tasks
            slot_ctr = 0
            pend_pair = None

            for t in range(NQB * 8 * NJ):
                qb, r = divmod(t, 8 * NJ)
                h, i = divmod(r, NJ)
                hh, p = h % 2, h // 2
                j = i
                # S^T tile for (qb, h, j): two bank-sized single groups
                s_ps = psp.tile([128, 1024], F32, name="s", tag="s", bufs=2)
                for sc in range(2):
                    nc.tensor.matmul(
                        s_ps[:, sc * 512:(sc + 1) * 512],
                        kt[p][hh * 64:hh * 64 + 64, j * 128:(j + 1) * 128],
                        qt[p][hh * 64:hh * 64 + 64,
                              qb * 1024 + sc * 512:qb * 1024 + (sc + 1) * 512],
                        start=True, stop=True)
                pt_t = ptp.tile([128, 1024], BF16, name="pt", tag="pt")
                nc.scalar.activation(pt_t[:], s_ps[:], AF.Exp, scale=0.125)
                cur_pts.append(pt_t)
                # drain up to 3 backlog attnV groups (prev head's); drain
                # BEFORE the fill so a head's groups start strictly after
                # its last iteration (all V'/P^T writers already emitted)
                for _ in range(3):
                    if not backlog:
                        break
                    pts_, bh, bqb, bqtl = backlog.popleft()
                    emit_attnv_qtl(pts_, bh, bqb, bqtl, slot_ctr % 2)
                    slot_ctr += 1
                    if bqtl == NQT - 1 and bh % 2 == 1:
                        pend_pair = (bh // 2, bqb)
                if pend_pair is not None and not backlog:
                    finish_pair(*pend_pair)
                    pend_pair = None
                if i == NJ - 1:
                    # head finished streaming: queue its attnV backlog
                    for qtl in range(NQT):
                        backlog.append((cur_pts, h, qb, qtl))
                    cur_pts = []
                # background projection / output-projection groups
                for fn, a in bg.get(t, ()):
                    fn(*a)

            # drain: last head's backlog, its pair, final outproj
            while backlog:
                pts_, bh, bqb, bqtl = backlog.popleft()
                emit_attnv_qtl(pts_, bh, bqb, bqtl, slot_ctr % 2)
                slot_ctr += 1
            finish_pair(3, NQB - 1)
            for qtl in range(NQT):
                emit_outproj_qtl(NQB - 1, qtl, tail=True)
    nc.finalize()
    return nc


def kernel(x, Wq, Wk, Wv, Wo, bo, _trace=False):
    npdt = mybir.dt.np(BF16)
    x = np.asarray(x, np.float32)
    bo = np.asarray(bo, np.float32)
    Wq, Wk, Wv = (np.asarray(a, np.float32) for a in (Wq, Wk, Wv))
    Wo = np.asarray(Wo, np.float32)

    if "nc" not in _CACHE:
        _CACHE["nc"] = build()
    nc = _CACHE["nc"]

    idon_in = np.concatenate(
        [np.eye(128, dtype=np.float32),
         np.ones((128, 1), np.float32)], axis=1).astype(npdt)
    in_maps = []
    for c in range(8):
        b, hb = c // 2, c % 2
        sl = slice(hb * HL, (hb + 1) * HL)
        xT = x[b].T  # [1024, 2048]
        xA = xT[:, 0:1024].reshape(NK, 128, 1024).transpose(1, 0, 2)
        xB = xT[:, 1024:2048].reshape(NK, 128, 1024).transpose(1, 0, 2)
        xtp = np.concatenate([xA.reshape(128, NK * 1024),
                              xB.reshape(128, NK * 1024)], axis=1)
        wqk = np.concatenate(
            [Wq[:, sl].reshape(NK, 128, HL),
             Wk[:, sl].reshape(NK, 128, HL)],
            axis=2).transpose(1, 0, 2).reshape(128, NK * 2 * HL)
        wqk0 = np.concatenate(
            [Wq[:, sl][:, 0:128].reshape(NK, 128, 128),
             Wk[:, sl][:, 0:128].reshape(NK, 128, 128)],
            axis=2).transpose(1, 0, 2).reshape(128, NK * 256)
        wvp = Wv[:, sl].reshape(NK, 128, HL).transpose(1, 0, 2).reshape(
            128, NK * HL)
        wop = Wo[sl, :].reshape(NP, 128, DIM).transpose(1, 0, 2).reshape(
            128, NP * DIM)
        in_maps.append({
            "xtp": np.ascontiguousarray(xtp).astype(npdt),
            "wqk": np.ascontiguousarray(wqk).astype(npdt),
            "wqk0": np.ascontiguousarray(wqk0).astype(npdt),
            "wvp": np.ascontiguousarray(wvp).astype(npdt),
            "wop": np.ascontiguousarray(wop).astype(npdt),
            "idon": idon_in,
        })
    res = run_bass_kernel_spmd(nc, in_maps, list(range(8)), trace=_trace)
    out = np.empty((4, N, DIM), np.float32)
    for b in range(4):
        out[b] = (res.results[2 * b]["out"].astype(np.float32)
                  + res.results[2 * b + 1]["out"].astype(np.float32) + bo)
    if _trace:
        return out, res
    return out


# revision 40
# speedup vs baseline: 1.3645x; 1.0108x over previous
"""MHA kernel for Trainium2, 8 NeuronCores.

Sharding: core c -> batch b = c//2, head-block hb = c%2 (8 of 16 heads).
Tensor-parallel within a batch: Wq/Wk/Wv column-sliced, Wo row-sliced;
each core emits a partial output [2048, 1024]; host sums the two partials
per batch and adds the bias.

All PE operands are bf16 (psum accumulation stays fp32); the host packs
x^T and the weights into partition-major blocks loaded in a few large DMAs.

Per-core dataflow (flash-attention style, q-block outer):
  QT/KT[p] = (Wq/Wk pair-slice)^T-proj          4x[128, 2048] bf16 sbuf
  V'[j]    = [x @ Wv | 1] per head              16x[128, 8x65] bf16 sbuf
  loop qb(2 q-blocks of 1024) x h(8) x j(16 kv-tiles):
    S^T  = KT_h[:,j].T @ QT_h[:,qb]  psum [128kv, 1024q]   (2 mm, K=64)
    P^T  = exp(0.125 S^T) on ACT  -> bf16 sbuf (persists one full head)
  attnV for head h runs as *backlog* during head h+1's stream: per q-tile
  qtl, one sequential psum accumulation group (its own bank, legal on HW):
    O[q, 65] = sum_j P^T_hj[:,qtl].T @ V'_j[:,h]   (16 mm, N=65;
               column 64 accumulates the softmax denominator)
  then per qtl: recip(den) + O*recip -> ONP bf16 (qt-major pair layout);
  per pair: PE-transpose ONP -> OT[p]; per qb: out[qt] = sum_p OT_p.T@Wo_p.
QKV projections and q-block 0's output projection are interleaved into the
main loop as background PE work (deadline-scheduled) so the PE never idles.
"""
import numpy as np
from collections import deque

import concourse.bacc as bacc
import concourse.bass as bass
import concourse.mybir as mybir
from concourse.tile import TileContext
from concourse.bass_utils import run_bass_kernel_spmd

F32 = mybir.dt.float32
BF16 = mybir.dt.bfloat16
AF = mybir.ActivationFunctionType

N = 2048       # tokens per batch
DIM = 1024     # model dim
HL = 512       # local inner (8 heads x 64)
NP = 4         # local head pairs
NJ = 16        # kv tiles of 128
NQB = 2        # q blocks of 1024
NQT = 8        # q tiles of 128 per q block
NK = 8         # contraction tiles of 128 over DIM

_CACHE = {}


def build():
    nc = bacc.Bacc(None, target_bir_lowering=False)
    # packed inputs: partition-major blocks.  x^T is packed as two column
    # halves (k-major within each) so the prelude only waits for the first.
    xt_d = nc.declare_dram_parameter("xtp", [128, NK * N], BF16, isOutput=False)
    wqk_d = nc.declare_dram_parameter("wqk", [128, NK * 2 * HL], BF16,
                                      isOutput=False)
    wqk0_d = nc.declare_dram_parameter("wqk0", [128, NK * 256], BF16,
                                       isOutput=False)
    wv_d = nc.declare_dram_parameter("wvp", [128, NK * HL], BF16,
                                     isOutput=False)
    wo_d = nc.declare_dram_parameter("wop", [128, NP * DIM], BF16,
                                     isOutput=False)
    idon_d = nc.declare_dram_parameter("idon", [128, 129], BF16, isOutput=False)
    out_d = nc.declare_dram_parameter("out", [N, DIM], BF16, isOutput=True)

    with TileContext(nc) as tc:
        with (
            tc.tile_pool(name="cn", bufs=1) as cn,       # constants + weights
            tc.tile_pool(name="act", bufs=1) as actp,    # QT/KT/V activations
            tc.tile_pool(name="pt", bufs=22) as ptp,     # P^T exp outputs
            tc.tile_pool(name="on", bufs=2) as onp_p,    # normalized O + O^T
            tc.tile_pool(name="sm", bufs=2) as smp,      # small: denr, out_sb
            tc.tile_pool(name="ps", bufs=1, space="PSUM") as psp,
        ):
            # ---- packed input DMAs, all on the SP queue in priority order
            idon = cn.tile([128, 129], BF16, name="idon", tag="idon")
            nc.sync.dma_start(out=idon[:], in_=idon_d[:])
            ident = idon[:, 0:128]
            wqk0s = cn.tile([128, NK * 256], BF16, name="wqk0s", tag="wqk0s")
            nc.sync.dma_start(out=wqk0s[:], in_=wqk0_d[:])
            xtsA = cn.tile([128, NK * 1024], BF16, name="xtsA", tag="xtsA")
            xtsB = cn.tile([128, NK * 1024], BF16, name="xtsB", tag="xtsB")
            for c in range(4):
                nc.sync.dma_start(out=xtsA[:, c * 2048:(c + 1) * 2048],
                                  in_=xt_d[:, c * 2048:(c + 1) * 2048])
            wvs = cn.tile([128, NK * HL], BF16, name="wvs", tag="wvs")
            nc.sync.dma_start(out=wvs[:], in_=wv_d[:])
            wqks = cn.tile([128, NK * 2 * HL], BF16, name="wqks", tag="wqks")
            for c in range(2):
                nc.sync.dma_start(out=wqks[:, c * 4096:(c + 1) * 4096],
                                  in_=wqk_d[:, c * 4096:(c + 1) * 4096])
            for c in range(2):
                nc.sync.dma_start(out=xtsB[:, c * 4096:(c + 1) * 4096],
                                  in_=xt_d[:, 8192 + c * 4096:8192 + (c + 1) * 4096])
            wos = cn.tile([128, NP * DIM], BF16, name="wos", tag="wos")
            nc.sync.dma_start(out=wos[:], in_=wo_d[:])

            def xt_cols(k, c0, w):
                # slice [c0, c0+w) of logical x^T k-chunk; never straddles
                # the 1024-column A/B boundary by construction
                if c0 < 1024:
                    assert c0 + w <= 1024
                    return xtsA[:, k * 1024 + c0:k * 1024 + c0 + w]
                return xtsB[:, k * 1024 + c0 - 1024:k * 1024 + c0 - 1024 + w]

            def wq(k):
                return wqks[:, k * 1024:k * 1024 + 512]

            def wk(k):
                return wqks[:, k * 1024 + 512:k * 1024 + 1024]

            def wv(k):
                return wvs[:, k * HL:(k + 1) * HL]

            def wo(p):
                return wos[:, p * DIM:(p + 1) * DIM]

            # ---- persistent activation tiles
            qt = [actp.tile([128, N], BF16, name=f"qt{p}", tag=f"qt{p}")
                  for p in range(NP)]
            kt = [actp.tile([128, N], BF16, name=f"kt{p}", tag=f"kt{p}")
                  for p in range(NP)]
            # V' = [V_h | 1] per head: 65 columns per head
            v = [actp.tile([128, 8 * 65], BF16, name=f"v{j}", tag=f"v{j}")
                 for j in range(NJ)]

            # ---- PE warmup: dummy matmuls on a memset tile keep the
            # p-state ramp busy while the input DMAs stream in.
            wsrc = cn.tile([128, 512], BF16, name="wsrc", tag="wsrc")
            nc.gpsimd.memset(wsrc[:], 0.0)
            # fill every V' tile with 1.0 once; the per-head 64-column
            # blocks are overwritten by the V projection copies, leaving
            # the denominator ones-columns (64::65) intact
            for j in range(NJ):
                nc.vector.memset(v[j][:], 1.0)
            warm = psp.tile([128, 512], F32, name="warm", tag="pj", bufs=2)

            def dummies(n):
                for _ in range(n):
                    nc.tensor.matmul(warm[:], wsrc[:, 0:128], wsrc[:],
                                     start=True, stop=True)

            # ---- background PE work: projection groups through the 2-bank
            # "pj" psum slots
            def emit_qkt_group(which, p, ncc, tag="pj", pre=False):
                if pre:
                    wf = (lambda k: wqk0s[:, k * 256:k * 256 + 128]) \
                        if which == "q" else \
                        (lambda k: wqk0s[:, k * 256 + 128:k * 256 + 256])
                else:
                    wf = wq if which == "q" else wk
                dst = qt[p] if which == "q" else kt[p]
                ps = psp.tile([128, 512], F32, name="pj", tag=tag, bufs=2)
                for k in range(NK):
                    w_k = wf(k) if pre else wf(k)[:, p * 128:(p + 1) * 128]
                    nc.tensor.matmul(
                        ps[:], w_k, xt_cols(k, ncc * 512, 512),
                        start=(k == 0), stop=(k == NK - 1))
                nc.vector.tensor_copy(
                    out=dst[:, ncc * 512:(ncc + 1) * 512], in_=ps[:])

            def emit_v_group(j, tag="pj"):
                ps = psp.tile([128, 512], F32, name="pj", tag=tag, bufs=2)
                for k in range(NK):
                    nc.tensor.matmul(
                        ps[:], xt_cols(k, j * 128, 128), wv(k),
                        start=(k == 0), stop=(k == NK - 1))
                # per-head rectangular copies into the 65-col V' layout
                for hh_ in range(8):
                    nc.vector.tensor_copy(
                        out=v[j][:, hh_ * 65:hh_ * 65 + 64],
                        in_=ps[:, hh_ * 64:(hh_ + 1) * 64])

            def emit_outproj_qtl(qb, qtl, tail=False):
                ots = [ot_tiles[(qb, p)] for p in range(NP)]
                r0 = qb * 1024 + qtl * 128
                osb = smp.tile([128, DIM], BF16, name="osb", tag="osb", bufs=4)
                if tail:
                    # s- and pj-tag psum banks are free after the last exp;
                    # rotate the dm-half groups through both for a 4-deep
                    # pipeline, copies alternating ACT/DVE
                    for dm in range(2):
                        tg = "s" if (qtl + dm) % 2 == 0 else "pj"
                        ps = psp.tile([128, 512], F32, name="tps", tag=tg,
                                      bufs=2)
                        for p in range(NP):
                            nc.tensor.matmul(
                                ps[:],
                                ots[p][:, qtl * 128:(qtl + 1) * 128],
                                wo(p)[:, dm * 512:(dm + 1) * 512],
                                start=(p == 0), stop=(p == NP - 1))
                        if dm == 0:
                            nc.scalar.copy(
                                out=osb[:, 0:512], in_=ps[:])
                        else:
                            nc.vector.tensor_copy(
                                out=osb[:, 512:1024], in_=ps[:])
                else:
                    for dm in range(2):
                        ps = psp.tile([128, 512], F32, name="pj", tag="pj",
                                      bufs=2)
                        for p in range(NP):
                            nc.tensor.matmul(
                                ps[:], ots[p][:, qtl * 128:(qtl + 1) * 128],
                                wo(p)[:, dm * 512:(dm + 1) * 512],
                                start=(p == 0), stop=(p == NP - 1))
                        nc.vector.tensor_copy(
                            out=osb[:, dm * 512:(dm + 1) * 512], in_=ps[:])
                nc.sync.dma_start(out=out_d[r0:r0 + 128, :], in_=osb[:])

            # ---- attnV backlog: head h's attention-times-V runs during
            # head h+1's S/exp stream, one legal psum group per q-tile.
            ot_tiles = {}
            onp_tiles = {}

            def emit_attnv_qtl(pts, h, qb, qtl, slot):
                o_t = psp.tile([128, 65], F32, name="oq", tag=f"o{slot}",
                               bufs=1)
                for j in range(NJ):
                    nc.tensor.matmul(
                        o_t[:],
                        pts[j][:, qtl * 128:(qtl + 1) * 128],
                        v[j][:, h * 65:h * 65 + 65],
                        start=(j == 0), stop=(j == NJ - 1))
                # normalize: ONP[:, qtl*128 + hh*64 + d] = O[:, d] / O[:, 64]
                p, hh = h // 2, h % 2
                if (p, qb) not in onp_tiles:
                    onp_tiles[(p, qb)] = onp_p.tile(
                        [128, 1024], BF16, name=f"onp{p}", tag=f"onp{p}")
                onp_t = onp_tiles[(p, qb)]
                denr = smp.tile([128, 1], F32, name="denr", tag="denr", bufs=4)
                with nc.allow_low_precision(reason="fp32 recip"):
                    nc.vector.reciprocal(denr[:], o_t[:, 64:65])
                nc.vector.tensor_scalar_mul(
                    onp_t[:, qtl * 128 + hh * 64:qtl * 128 + hh * 64 + 64],
                    o_t[:, 0:64], denr[:, 0:1])

            def finish_pair(p, qb):
                # PE-transpose ONP -> OT_p [128 d, 1024 q] for this q block
                onp_t = onp_tiles[(p, qb)]
                tp = psp.tile([128, 1024], BF16, name="tp", tag="pj", bufs=2)
                for qtl in range(NQT):
                    nc.tensor.transpose(
                        tp[:, qtl * 128:(qtl + 1) * 128],
                        onp_t[:, qtl * 128:(qtl + 1) * 128], ident)
                ot = onp_p.tile([128, 1024], BF16, name=f"ot{p}", tag=f"ot{p}")
                nc.vector.tensor_copy(out=ot[:], in_=tp[:])
                ot_tiles[(qb, p)] = ot

            # background emission schedule: iter t = (qb*8 + h)*16 + j.
            # (deadline_iter, earliest_iter, fn, args)
            groups = []
            for j in range(1, NJ):
                # all V' tiles are read by head-0's backlog from iter 16
                groups.append((j, 0, emit_v_group, (j,)))
            for p in range(NP):
                for c in range(4):  # KT chunk c first read at t=32p+4c
                    if p == 0 and c == 0:
                        continue  # prelude
                    groups.append((32 * p + 4 * c - 3, 0,
                                   emit_qkt_group, ("k", p, c)))
                for ncc in range(4):
                    if p == 0 and ncc in (0, 1):
                        continue  # prelude
                    first = 128 * (ncc // 2) + 32 * p
                    groups.append((first - 4, 0, emit_qkt_group, ("q", p, ncc)))
            for qtl in range(NQT):  # q-block 0 outproj during q-block 1
                groups.append((250, 140 + 2 * qtl, emit_outproj_qtl, (0, qtl)))

            q_bg = deque(sorted(groups, key=lambda g: (g[0], g[1])))
            bg = {}
            last_emit = -10
            for tt in range(NQB * 8 * NJ):
                while q_bg and q_bg[0][0] <= tt:
                    g = q_bg.popleft()
                    bg.setdefault(tt, []).append((g[2], g[3]))
                    last_emit = tt
                if q_bg and tt - last_emit >= 2 and q_bg[0][1] <= tt:
                    g = q_bg.popleft()
                    bg.setdefault(tt, []).append((g[2], g[3]))
                    last_emit = tt

            # ---- prelude: just enough for S(h0, j0)
            dummies(8)
            emit_qkt_group("q", 0, 0, tag="s", pre=True)
            dummies(2)
            emit_qkt_group("q", 0, 1, tag="s", pre=True)
            dummies(2)
            emit_qkt_group("k", 0, 0, tag="pj", pre=True)
            emit_v_group(0)

            # ---- main loop
            cur_pts = []          # P^T tiles of the in-flight head
            backlog = deque()     # (pts, h, qb, qtl) attnV tasks
            slot_ctr = 0
            pend_pair = None

            for t in range(NQB * 8 * NJ):
                qb, r = divmod(t, 8 * NJ)
                h, i = divmod(r, NJ)
                hh, p = h % 2, h // 2
                j = i
                # S^T tile for (qb, h, j): two bank-sized single groups
                s_ps = psp.tile([128, 1024], F32, name="s", tag="s", bufs=2)
                for sc in range(2):
                    nc.tensor.matmul(
                        s_ps[:, sc * 512:(sc + 1) * 512],
                        kt[p][hh * 64:hh * 64 + 64, j * 128:(j + 1) * 128],
                        qt[p][hh * 64:hh * 64 + 64,
                              qb * 1024 + sc * 512:qb * 1024 + (sc + 1) * 512],
                        start=True, stop=True)
                pt_t = ptp.tile([128, 1024], BF16, name="pt", tag="pt")
                nc.scalar.activation(pt_t[:], s_ps[:], AF.Exp, scale=0.125)
                cur_pts.append(pt_t)
                # drain up to 3 backlog attnV groups (prev head's); drain
                # BEFORE the fill so a head's groups start strictly after
                # its last iteration (all V'/P^T writers already emitted)
                for _ in range(3):
                    if not backlog:
                        break
                    pts_, bh, bqb, bqtl = backlog.popleft()
                    emit_attnv_qtl(pts_, bh, bqb, bqtl, slot_ctr % 2)
                    slot_ctr += 1
                    if bqtl == NQT - 1 and bh % 2 == 1:
                        pend_pair = (bh // 2, bqb)
                if pend_pair is not None and not backlog:
                    finish_pair(*pend_pair)
                    pend_pair = None
                if i == NJ - 1:
                    # head finished streaming: queue its attnV backlog
                    for qtl in range(NQT):
                        backlog.append((cur_pts, h, qb, qtl))
                    cur_pts = []
                # background projection / output-projection groups
                for fn, a in bg.get(t, ()):
                    fn(*a)

            # drain: last head's backlog, its pair, final outproj
            while backlog:
                pts_, bh, bqb, bqtl = backlog.popleft()
                emit_attnv_qtl(pts_, bh, bqb, bqtl, slot_ctr % 2)
                slot_ctr += 1
            finish_pair(3, NQB - 1)
            for qtl in range(NQT):
                emit_outproj_qtl(NQB - 1, qtl, tail=True)
    nc.finalize()
    return nc


def kernel(x, Wq, Wk, Wv, Wo, bo, _trace=False):
    npdt = mybir.dt.np(BF16)
    x = np.asarray(x, np.float32)
    bo = np.asarray(bo, np.float32)
    Wq, Wk, Wv = (np.asarray(a, np.float32) for a in (Wq, Wk, Wv))
    Wo = np.asarray(Wo, np.float32)

    if "nc" not in _CACHE:
        _CACHE["nc"] = build()
    nc = _CACHE["nc"]

    idon_in = np.concatenate(
        [np.eye(128, dtype=np.float32),
         np.ones((128, 1), np.float32)], axis=1).astype(npdt)
    in_maps = []
    for c in range(8):
        b, hb = c // 2, c % 2
        sl = slice(hb * HL, (hb + 1) * HL)
        xT = x[b].T  # [1024, 2048]
        xA = xT[:, 0:1024].reshape(NK, 128, 1024).transpose(1, 0, 2)
        xB = xT[:, 1024:2048].reshape(NK, 128, 1024).transpose(1, 0, 2)
        xtp = np.concatenate([xA.reshape(128, NK * 1024),
                              xB.reshape(128, NK * 1024)], axis=1)
        wqk = np.concatenate(
            [Wq[:, sl].reshape(NK, 128, HL),
             Wk[:, sl].reshape(NK, 128, HL)],
            axis=2).transpose(1, 0, 2).reshape(128, NK * 2 * HL)
        wqk0 = np.concatenate(
            [Wq[:, sl][:, 0:128].reshape(NK, 128, 128),
             Wk[:, sl][:, 0:128].reshape(NK, 128, 128)],
            axis=2).transpose(1, 0, 2).reshape(128, NK * 256)
        wvp = Wv[:, sl].reshape(NK, 128, HL).transpose(1, 0, 2).reshape(
            128, NK * HL)
        wop = Wo[sl, :].reshape(NP, 128, DIM).transpose(1, 0, 2).reshape(
            128, NP * DIM)
        in_maps.append({
            "xtp": np.ascontiguousarray(xtp).astype(npdt),
            "wqk": np.ascontiguousarray(wqk).astype(npdt),
            "wqk0": np.ascontiguousarray(wqk0).astype(npdt),
            "wvp": np.ascontiguousarray(wvp).astype(npdt),
            "wop": np.ascontiguousarray(wop).astype(npdt),
            "idon": idon_in,
        })
    res = run_bass_kernel_spmd(nc, in_maps, list(range(8)), trace=_trace)
    out = np.empty((4, N, DIM), np.float32)
    for b in range(4):
        out[b] = (res.results[2 * b]["out"].astype(np.float32)
                  + res.results[2 * b + 1]["out"].astype(np.float32) + bo)
    if _trace:
        return out, res
    return out


# revision 46
# speedup vs baseline: 1.4519x; 1.0641x over previous
"""MHA kernel for Trainium2, 8 NeuronCores.

Sharding: core c -> batch b = c//2, head-block hb = c%2 (8 of 16 heads).
Tensor-parallel within a batch: Wq/Wk/Wv column-sliced, Wo row-sliced;
each core emits a partial output [2048, 1024]; host sums the two partials
per batch and adds the bias.

All PE operands are bf16 (psum accumulation stays fp32); the host packs
x^T and the weights into partition-major blocks loaded in a few large DMAs.

Per-core dataflow (flash-attention style, q-block outer):
  QT/KT[p] = (Wq/Wk pair-slice)^T-proj          4x[128, 2048] bf16 sbuf
  V'[j]    = [x @ Wv | 1] per head              16x[128, 8x65] bf16 sbuf
  loop qb(2 q-blocks of 1024) x h(8) x j(16 kv-tiles):
    S^T  = KT_h[:,j].T @ QT_h[:,qb]  psum [128kv, 1024q]   (2 mm, K=64)
    P^T  = exp(0.125 S^T) on ACT  -> bf16 sbuf (persists one full head)
  attnV for head h runs as *backlog* during head h+1's stream: per q-tile
  qtl, one sequential psum accumulation group (its own bank, legal on HW):
    O[q, 65] = sum_j P^T_hj[:,qtl].T @ V'_j[:,h]   (16 mm, N=65;
               column 64 accumulates the softmax denominator)
  then per qtl: recip(den) + O*recip -> ONP bf16 (qt-major pair layout);
  per pair: PE-transpose ONP -> OT[p]; per qb: out[qt] = sum_p OT_p.T@Wo_p.
QKV projections and q-block 0's output projection are interleaved into the
main loop as background PE work (deadline-scheduled) so the PE never idles.
"""
import numpy as np
from collections import deque

import concourse.bacc as bacc
import concourse.bass as bass
import concourse.mybir as mybir
from concourse.tile import TileContext
from concourse.bass_utils import run_bass_kernel_spmd

F32 = mybir.dt.float32
BF16 = mybir.dt.bfloat16
AF = mybir.ActivationFunctionType

N = 2048       # tokens per batch
DIM = 1024     # model dim
HL = 512       # local inner (8 heads x 64)
NP = 4         # local head pairs
NJ = 16        # kv tiles of 128
NQB = 2        # q blocks of 1024
NQT = 8        # q tiles of 128 per q block
NK = 8         # contraction tiles of 128 over DIM

_CACHE = {}


def build():
    nc = bacc.Bacc(None, target_bir_lowering=False)
    # packed inputs: partition-major blocks.  x^T is packed as two column
    # halves (k-major within each) so the prelude only waits for the first.
    xt_d = nc.declare_dram_parameter("xtp", [128, NK * N], BF16, isOutput=False)
    wqk_d = nc.declare_dram_parameter("wqk", [128, NK * 2 * HL], BF16,
                                      isOutput=False)
    wqk0_d = nc.declare_dram_parameter("wqk0", [128, NK * 256], BF16,
                                       isOutput=False)
    wv_d = nc.declare_dram_parameter("wvp", [128, NK * HL], BF16,
                                     isOutput=False)
    wo_d = nc.declare_dram_parameter("wop", [128, NP * DIM], BF16,
                                     isOutput=False)
    idon_d = nc.declare_dram_parameter("idon", [128, 129], BF16, isOutput=False)
    out_d = nc.declare_dram_parameter("out", [N, DIM], BF16, isOutput=True)

    with TileContext(nc) as tc:
        with (
            tc.tile_pool(name="cn", bufs=1) as cn,       # constants + weights
            tc.tile_pool(name="act", bufs=1) as actp,    # QT/KT/V activations
            tc.tile_pool(name="pt", bufs=22) as ptp,     # P^T exp outputs
            tc.tile_pool(name="on", bufs=2) as onp_p,    # normalized O + O^T
            tc.tile_pool(name="sm", bufs=2) as smp,      # small: denr, out_sb
            tc.tile_pool(name="ps", bufs=1, space="PSUM") as psp,
        ):
            # ---- packed input DMAs, all on the SP queue in priority order
            idon = cn.tile([128, 129], BF16, name="idon", tag="idon")
            nc.sync.dma_start(out=idon[:], in_=idon_d[:])
            ident = idon[:, 0:128]
            wqk0s = cn.tile([128, NK * 256], BF16, name="wqk0s", tag="wqk0s")
            nc.sync.dma_start(out=wqk0s[:], in_=wqk0_d[:])
            xtsA = cn.tile([128, NK * 1024], BF16, name="xtsA", tag="xtsA")
            xtsB = cn.tile([128, NK * 1024], BF16, name="xtsB", tag="xtsB")
            for c in range(4):
                nc.sync.dma_start(out=xtsA[:, c * 2048:(c + 1) * 2048],
                                  in_=xt_d[:, c * 2048:(c + 1) * 2048])
            wvs = cn.tile([128, NK * HL], BF16, name="wvs", tag="wvs")
            nc.sync.dma_start(out=wvs[:], in_=wv_d[:])
            wqks = cn.tile([128, NK * 2 * HL], BF16, name="wqks", tag="wqks")
            for c in range(2):
                nc.sync.dma_start(out=wqks[:, c * 4096:(c + 1) * 4096],
                                  in_=wqk_d[:, c * 4096:(c + 1) * 4096])
            for c in range(2):
                nc.sync.dma_start(out=xtsB[:, c * 4096:(c + 1) * 4096],
                                  in_=xt_d[:, 8192 + c * 4096:8192 + (c + 1) * 4096])
            wos = cn.tile([128, NP * DIM], BF16, name="wos", tag="wos")
            nc.sync.dma_start(out=wos[:], in_=wo_d[:])

            def xt_cols(k, c0, w):
                # slice [c0, c0+w) of logical x^T k-chunk; never straddles
                # the 1024-column A/B boundary by construction
                if c0 < 1024:
                    assert c0 + w <= 1024
                    return xtsA[:, k * 1024 + c0:k * 1024 + c0 + w]
                return xtsB[:, k * 1024 + c0 - 1024:k * 1024 + c0 - 1024 + w]

            def wq(k):
                return wqks[:, k * 1024:k * 1024 + 512]

            def wk(k):
                return wqks[:, k * 1024 + 512:k * 1024 + 1024]

            def wv(k):
                return wvs[:, k * HL:(k + 1) * HL]

            def wo(p):
                return wos[:, p * DIM:(p + 1) * DIM]

            # ---- persistent activation tiles
            qt = [actp.tile([128, N], BF16, name=f"qt{p}", tag=f"qt{p}")
                  for p in range(NP)]
            kt = [actp.tile([128, N], BF16, name=f"kt{p}", tag=f"kt{p}")
                  for p in range(NP)]
            # V' = [V_h | 1] per head: 65 columns per head
            v = [actp.tile([128, 8 * 65], BF16, name=f"v{j}", tag=f"v{j}")
                 for j in range(NJ)]

            # ---- PE warmup: dummy matmuls on a memset tile keep the
            # p-state ramp busy while the input DMAs stream in.
            wsrc = cn.tile([128, 512], BF16, name="wsrc", tag="wsrc")
            nc.gpsimd.memset(wsrc[:], 0.0)
            # fill every V' tile with 1.0 once; the per-head 64-column
            # blocks are overwritten by the V projection copies, leaving
            # the denominator ones-columns (64::65) intact
            for j in range(NJ):
                nc.vector.memset(v[j][:], 1.0)
            warm = psp.tile([128, 512], F32, name="warm", tag="pj", bufs=2)

            def dummies(n):
                for _ in range(n):
                    nc.tensor.matmul(warm[:], wsrc[:, 0:128], wsrc[:],
                                     start=True, stop=True)

            # ---- background PE work: projection groups through the 2-bank
            # "pj" psum slots
            def emit_qkt_group(which, p, ncc, tag="pj", pre=False):
                if pre:
                    wf = (lambda k: wqk0s[:, k * 256:k * 256 + 128]) \
                        if which == "q" else \
                        (lambda k: wqk0s[:, k * 256 + 128:k * 256 + 256])
                else:
                    wf = wq if which == "q" else wk
                dst = qt[p] if which == "q" else kt[p]
                ps = psp.tile([128, 512], F32, name="pj", tag=tag, bufs=2)
                for k in range(NK):
                    w_k = wf(k) if pre else wf(k)[:, p * 128:(p + 1) * 128]
                    nc.tensor.matmul(
                        ps[:], w_k, xt_cols(k, ncc * 512, 512),
                        start=(k == 0), stop=(k == NK - 1))
                nc.vector.tensor_copy(
                    out=dst[:, ncc * 512:(ncc + 1) * 512], in_=ps[:])

            def emit_v_group(j, tag="pj"):
                ps = psp.tile([128, 512], F32, name="pj", tag=tag, bufs=2)
                for k in range(NK):
                    nc.tensor.matmul(
                        ps[:], xt_cols(k, j * 128, 128), wv(k),
                        start=(k == 0), stop=(k == NK - 1))
                # per-head rectangular copies into the 65-col V' layout
                for hh_ in range(8):
                    nc.vector.tensor_copy(
                        out=v[j][:, hh_ * 65:hh_ * 65 + 64],
                        in_=ps[:, hh_ * 64:(hh_ + 1) * 64])

            def emit_outproj_qtl(qb, qtl, tail=False):
                ots = [ot_tiles[(qb, p)] for p in range(NP)]
                r0 = qb * 1024 + qtl * 128
                osb = smp.tile([128, DIM], BF16, name="osb", tag="osb", bufs=4)
                if tail:
                    # s- and pj-tag psum banks are free after the last exp;
                    # rotate the dm-half groups through both for a 4-deep
                    # pipeline, copies alternating ACT/DVE
                    for dm in range(2):
                        tg = "s" if (qtl + dm) % 2 == 0 else "pj"
                        ps = psp.tile([128, 512], F32, name="tps", tag=tg,
                                      bufs=2)
                        for p in range(NP):
                            nc.tensor.matmul(
                                ps[:],
                                ots[p][:, qtl * 128:(qtl + 1) * 128],
                                wo(p)[:, dm * 512:(dm + 1) * 512],
                                start=(p == 0), stop=(p == NP - 1))
                        if dm == 0:
                            nc.scalar.copy(
                                out=osb[:, 0:512], in_=ps[:])
                        else:
                            nc.vector.tensor_copy(
                                out=osb[:, 512:1024], in_=ps[:])
                else:
                    for dm in range(2):
                        ps = psp.tile([128, 512], F32, name="pj", tag="pj",
                                      bufs=2)
                        for p in range(NP):
                            nc.tensor.matmul(
                                ps[:], ots[p][:, qtl * 128:(qtl + 1) * 128],
                                wo(p)[:, dm * 512:(dm + 1) * 512],
                                start=(p == 0), stop=(p == NP - 1))
                        nc.vector.tensor_copy(
                            out=osb[:, dm * 512:(dm + 1) * 512], in_=ps[:])
                nc.sync.dma_start(out=out_d[r0:r0 + 128, :], in_=osb[:])

            # ---- attnV backlog: head h's attention-times-V runs during
            # head h+1's S/exp stream, one legal psum group per q-tile.
            ot_tiles = {}
            onp_tiles = {}

            def emit_attnv_qtl(pts, h, qb, qtl, slot):
                o_t = psp.tile([128, 65], F32, name="oq", tag=f"o{slot}",
                               bufs=1)
                for j in range(NJ):
                    nc.tensor.matmul(
                        o_t[:],
                        pts[j][:, qtl * 128:(qtl + 1) * 128],
                        v[j][:, h * 65:h * 65 + 65],
                        start=(j == 0), stop=(j == NJ - 1))
                # normalize: ONP[:, qtl*128 + hh*64 + d] = O[:, d] / O[:, 64]
                p, hh = h // 2, h % 2
                if (p, qb) not in onp_tiles:
                    onp_tiles[(p, qb)] = onp_p.tile(
                        [128, 1024], BF16, name=f"onp{p}", tag=f"onp{p}")
                onp_t = onp_tiles[(p, qb)]
                denr = smp.tile([128, 1], F32, name="denr", tag="denr", bufs=4)
                with nc.allow_low_precision(reason="fp32 recip"):
                    nc.vector.reciprocal(denr[:], o_t[:, 64:65])
                nc.vector.tensor_scalar_mul(
                    onp_t[:, qtl * 128 + hh * 64:qtl * 128 + hh * 64 + 64],
                    o_t[:, 0:64], denr[:, 0:1])

            def finish_pair(p, qb):
                # PE-transpose ONP -> OT_p [128 d, 1024 q] for this q block
                onp_t = onp_tiles[(p, qb)]
                tp = psp.tile([128, 1024], BF16, name="tp", tag="pj", bufs=2)
                for qtl in range(NQT):
                    nc.tensor.transpose(
                        tp[:, qtl * 128:(qtl + 1) * 128],
                        onp_t[:, qtl * 128:(qtl + 1) * 128], ident)
                ot = onp_p.tile([128, 1024], BF16, name=f"ot{p}", tag=f"ot{p}")
                nc.vector.tensor_copy(out=ot[:], in_=tp[:])
                ot_tiles[(qb, p)] = ot

            # background emission schedule: iter t = (qb*8 + h)*16 + j.
            # (deadline_iter, earliest_iter, fn, args)
            groups = []
            for j in range(1, NJ):
                # all V' tiles are read by head-0's backlog from iter 16
                groups.append((j, 0, emit_v_group, (j,)))
            for p in range(NP):
                for c in range(4):  # KT chunk c first read at t=32p+4c
                    if p == 0 and c == 0:
                        continue  # prelude
                    groups.append((32 * p + 4 * c - 3, 0,
                                   emit_qkt_group, ("k", p, c)))
                for ncc in range(4):
                    if p == 0 and ncc in (0, 1):
                        continue  # prelude
                    first = 128 * (ncc // 2) + 32 * p
                    groups.append((first - 4, 0, emit_qkt_group, ("q", p, ncc)))
            for qtl in range(NQT):  # q-block 0 outproj during q-block 1
                groups.append((250, 140 + 2 * qtl, emit_outproj_qtl, (0, qtl)))

            q_bg = deque(sorted(groups, key=lambda g: (g[0], g[1])))
            bg = {}
            last_emit = -10
            for tt in range(NQB * 8 * NJ):
                while q_bg and q_bg[0][0] <= tt:
                    g = q_bg.popleft()
                    bg.setdefault(tt, []).append((g[2], g[3]))
                    last_emit = tt
                if q_bg and tt - last_emit >= 9 and q_bg[0][1] <= tt:
                    g = q_bg.popleft()
                    bg.setdefault(tt, []).append((g[2], g[3]))
                    last_emit = tt

            # ---- prelude: just enough for S(h0, j0)
            dummies(8)
            emit_qkt_group("q", 0, 0, tag="s", pre=True)
            dummies(2)
            emit_qkt_group("q", 0, 1, tag="s", pre=True)
            dummies(2)
            emit_qkt_group("k", 0, 0, tag="pj", pre=True)
            emit_v_group(0)

            # ---- main loop
            cur_pts = []          # P^T tiles of the in-flight head
            backlog = deque()     # (pts, h, qb, qtl) attnV tasks
            slot_ctr = 0
            pend_pair = None

            for t in range(NQB * 8 * NJ):
                qb, r = divmod(t, 8 * NJ)
                h, i = divmod(r, NJ)
                hh, p = h % 2, h // 2
                j = i
                # S^T tile for (qb, h, j): two bank-sized single groups
                s_ps = psp.tile([128, 1024], F32, name="s", tag="s", bufs=2)
                for sc in range(2):
                    nc.tensor.matmul(
                        s_ps[:, sc * 512:(sc + 1) * 512],
                        kt[p][hh * 64:hh * 64 + 64, j * 128:(j + 1) * 128],
                        qt[p][hh * 64:hh * 64 + 64,
                              qb * 1024 + sc * 512:qb * 1024 + (sc + 1) * 512],
                        start=True, stop=True)
                pt_t = ptp.tile([128, 1024], BF16, name="pt", tag="pt")
                nc.scalar.activation(pt_t[:], s_ps[:], AF.Exp, scale=0.125)
                cur_pts.append(pt_t)
                # drain up to 3 backlog attnV groups (prev head's); drain
                # BEFORE the fill so a head's groups start strictly after
                # its last iteration (all V'/P^T writers already emitted)
                for _ in range(3):
                    if not backlog:
                        break
                    pts_, bh, bqb, bqtl = backlog.popleft()
                    emit_attnv_qtl(pts_, bh, bqb, bqtl, slot_ctr % 2)
                    slot_ctr += 1
                    if bqtl == NQT - 1 and bh % 2 == 1:
                        pend_pair = (bh // 2, bqb)
                if pend_pair is not None and not backlog and i >= 5:
                    # deferred so the DVE norm chain has fully drained and
                    # the transposes never stall the in-order PE queue
                    finish_pair(*pend_pair)
                    pend_pair = None
                if i == NJ - 1:
                    # head finished streaming: queue its attnV backlog
                    for qtl in range(NQT):
                        backlog.append((cur_pts, h, qb, qtl))
                    cur_pts = []
                # background projection / output-projection groups
                for fn, a in bg.get(t, ()):
                    fn(*a)

            # drain: last head's backlog, its pair, final outproj
            while backlog:
                pts_, bh, bqb, bqtl = backlog.popleft()
                emit_attnv_qtl(pts_, bh, bqb, bqtl, slot_ctr % 2)
                slot_ctr += 1
            finish_pair(3, NQB - 1)
            for qtl in range(NQT):
                emit_outproj_qtl(NQB - 1, qtl, tail=True)
    nc.finalize()
    return nc


def kernel(x, Wq, Wk, Wv, Wo, bo, _trace=False):
    npdt = mybir.dt.np(BF16)
    x = np.asarray(x, np.float32)
    bo = np.asarray(bo, np.float32)
    Wq, Wk, Wv = (np.asarray(a, np.float32) for a in (Wq, Wk, Wv))
    Wo = np.asarray(Wo, np.float32)

    if "nc" not in _CACHE:
        _CACHE["nc"] = build()
    nc = _CACHE["nc"]

    idon_in = np.concatenate(
        [np.eye(128, dtype=np.float32),
         np.ones((128, 1), np.float32)], axis=1).astype(npdt)
    in_maps = []
    for c in range(8):
        b, hb = c // 2, c % 2
        sl = slice(hb * HL, (hb + 1) * HL)
        xT = x[b].T  # [1024, 2048]
        xA = xT[:, 0:1024].reshape(NK, 128, 1024).transpose(1, 0, 2)
        xB = xT[:, 1024:2048].reshape(NK, 128, 1024).transpose(1, 0, 2)
        xtp = np.concatenate([xA.reshape(128, NK * 1024),
                              xB.reshape(128, NK * 1024)], axis=1)
        wqk = np.concatenate(
            [Wq[:, sl].reshape(NK, 128, HL),
             Wk[:, sl].reshape(NK, 128, HL)],
            axis=2).transpose(1, 0, 2).reshape(128, NK * 2 * HL)
        wqk0 = np.concatenate(
            [Wq[:, sl][:, 0:128].reshape(NK, 128, 128),
             Wk[:, sl][:, 0:128].reshape(NK, 128, 128)],
            axis=2).transpose(1, 0, 2).reshape(128, NK * 256)
        wvp = Wv[:, sl].reshape(NK, 128, HL).transpose(1, 0, 2).reshape(
            128, NK * HL)
        wop = Wo[sl, :].reshape(NP, 128, DIM).transpose(1, 0, 2).reshape(
            128, NP * DIM)
        in_maps.append({
            "xtp": np.ascontiguousarray(xtp).astype(npdt),
            "wqk": np.ascontiguousarray(wqk).astype(npdt),
            "wqk0": np.ascontiguousarray(wqk0).astype(npdt),
            "wvp": np.ascontiguousarray(wvp).astype(npdt),
            "wop": np.ascontiguousarray(wop).astype(npdt),
            "idon": idon_in,
        })
    res = run_bass_kernel_spmd(nc, in_maps, list(range(8)), trace=_trace)
    out = np.empty((4, N, DIM), np.float32)
    for b in range(4):
        out[b] = (res.results[2 * b]["out"].astype(np.float32)
                  + res.results[2 * b + 1]["out"].astype(np.float32) + bo)
    if _trace:
        return out, res
    return out


# revision 50
# speedup vs baseline: 1.4844x; 1.0224x over previous
"""MHA kernel for Trainium2, 8 NeuronCores.

Sharding: core c -> batch b = c//2, head-block hb = c%2 (8 of 16 heads).
Tensor-parallel within a batch: Wq/Wk/Wv column-sliced, Wo row-sliced;
each core emits a partial output [2048, 1024]; host sums the two partials
per batch and adds the bias.

All PE operands are bf16 (psum accumulation stays fp32); the host packs
x^T and the weights into partition-major blocks loaded in a few large DMAs.

Per-core dataflow (flash-attention style, q-block outer):
  QT/KT[p] = (Wq/Wk pair-slice)^T-proj          4x[128, 2048] bf16 sbuf
  V'[j]    = [x @ Wv | 1] per head              16x[128, 8x65] bf16 sbuf
  loop qb(2 q-blocks of 1024) x h(8) x j(16 kv-tiles):
    S^T  = KT_h[:,j].T @ QT_h[:,qb]  psum [128kv, 1024q]   (2 mm, K=64)
    P^T  = exp(0.125 S^T) on ACT  -> bf16 sbuf (persists one full head)
  attnV for head h runs as *backlog* during head h+1's stream: per q-tile
  qtl, one sequential psum accumulation group (its own bank, legal on HW):
    O[q, 65] = sum_j P^T_hj[:,qtl].T @ V'_j[:,h]   (16 mm, N=65;
               column 64 accumulates the softmax denominator)
  then per qtl: recip(den) + O*recip -> ONP bf16 (qt-major pair layout);
  per pair: PE-transpose ONP -> OT[p]; per qb: out[qt] = sum_p OT_p.T@Wo_p.
QKV projections and q-block 0's output projection are interleaved into the
main loop as background PE work (deadline-scheduled) so the PE never idles.
"""
import numpy as np
from collections import deque

import concourse.bacc as bacc
import concourse.bass as bass
import concourse.mybir as mybir
from concourse.tile import TileContext
from concourse.bass_utils import run_bass_kernel_spmd

F32 = mybir.dt.float32
BF16 = mybir.dt.bfloat16
AF = mybir.ActivationFunctionType

N = 2048       # tokens per batch
DIM = 1024     # model dim
HL = 512       # local inner (8 heads x 64)
NP = 4         # local head pairs
NJ = 16        # kv tiles of 128
NQB = 2        # q blocks of 1024
NQT = 8        # q tiles of 128 per q block
NK = 8         # contraction tiles of 128 over DIM

_CACHE = {}


def build():
    nc = bacc.Bacc(None, target_bir_lowering=False)
    # packed inputs: partition-major blocks.  x^T is packed as two column
    # halves (k-major within each) so the prelude only waits for the first.
    xt_d = nc.declare_dram_parameter("xtp", [128, NK * N], BF16, isOutput=False)
    wqk_d = nc.declare_dram_parameter("wqk", [128, NK * 2 * HL], BF16,
                                      isOutput=False)
    wqk0_d = nc.declare_dram_parameter("wqk0", [128, NK * 256], BF16,
                                       isOutput=False)
    wv_d = nc.declare_dram_parameter("wvp", [128, NK * HL], BF16,
                                     isOutput=False)
    wo_d = nc.declare_dram_parameter("wop", [128, NP * DIM], BF16,
                                     isOutput=False)
    idon_d = nc.declare_dram_parameter("idon", [128, 129], BF16, isOutput=False)
    out_d = nc.declare_dram_parameter("out", [N, DIM], BF16, isOutput=True)

    with TileContext(nc) as tc:
        with (
            tc.tile_pool(name="cn", bufs=1) as cn,       # constants + weights
            tc.tile_pool(name="act", bufs=1) as actp,    # QT/KT/V activations
            tc.tile_pool(name="pt", bufs=22) as ptp,     # P^T exp outputs
            tc.tile_pool(name="on", bufs=2) as onp_p,    # normalized O + O^T
            tc.tile_pool(name="sm", bufs=2) as smp,      # small: denr, out_sb
            tc.tile_pool(name="ps", bufs=1, space="PSUM") as psp,
        ):
            # ---- packed input DMAs, all on the SP queue in priority order
            idon = cn.tile([128, 129], BF16, name="idon", tag="idon")
            nc.sync.dma_start(out=idon[:], in_=idon_d[:])
            ident = idon[:, 0:128]
            wqk0s = cn.tile([128, NK * 256], BF16, name="wqk0s", tag="wqk0s")
            nc.sync.dma_start(out=wqk0s[:], in_=wqk0_d[:])
            xtsA = cn.tile([128, NK * 1024], BF16, name="xtsA", tag="xtsA")
            xtsB = cn.tile([128, NK * 1024], BF16, name="xtsB", tag="xtsB")
            for c in range(4):
                nc.sync.dma_start(out=xtsA[:, c * 2048:(c + 1) * 2048],
                                  in_=xt_d[:, c * 2048:(c + 1) * 2048])
            wvs = cn.tile([128, NK * HL], BF16, name="wvs", tag="wvs")
            nc.sync.dma_start(out=wvs[:], in_=wv_d[:])
            wqks = cn.tile([128, NK * 2 * HL], BF16, name="wqks", tag="wqks")
            for c in range(2):
                nc.sync.dma_start(out=wqks[:, c * 4096:(c + 1) * 4096],
                                  in_=wqk_d[:, c * 4096:(c + 1) * 4096])
            for c in range(2):
                nc.sync.dma_start(out=xtsB[:, c * 4096:(c + 1) * 4096],
                                  in_=xt_d[:, 8192 + c * 4096:8192 + (c + 1) * 4096])
            wos = cn.tile([128, NP * DIM], BF16, name="wos", tag="wos")
            nc.sync.dma_start(out=wos[:], in_=wo_d[:])

            def xt_cols(k, c0, w):
                # slice [c0, c0+w) of logical x^T k-chunk; never straddles
                # the 1024-column A/B boundary by construction
                if c0 < 1024:
                    assert c0 + w <= 1024
                    return xtsA[:, k * 1024 + c0:k * 1024 + c0 + w]
                return xtsB[:, k * 1024 + c0 - 1024:k * 1024 + c0 - 1024 + w]

            def wq(k):
                return wqks[:, k * 1024:k * 1024 + 512]

            def wk(k):
                return wqks[:, k * 1024 + 512:k * 1024 + 1024]

            def wv(k):
                return wvs[:, k * HL:(k + 1) * HL]

            def wo(p):
                return wos[:, p * DIM:(p + 1) * DIM]

            # ---- persistent activation tiles
            qt = [actp.tile([128, N], BF16, name=f"qt{p}", tag=f"qt{p}")
                  for p in range(NP)]
            kt = [actp.tile([128, N], BF16, name=f"kt{p}", tag=f"kt{p}")
                  for p in range(NP)]
            # V' = [V_h | 1] per head: 65 columns per head
            v = [actp.tile([128, 8 * 65], BF16, name=f"v{j}", tag=f"v{j}")
                 for j in range(NJ)]

            # ---- PE warmup: dummy matmuls on a memset tile keep the
            # p-state ramp busy while the input DMAs stream in.
            wsrc = cn.tile([128, 512], BF16, name="wsrc", tag="wsrc")
            nc.gpsimd.memset(wsrc[:], 0.0)
            # fill every V' tile with 1.0 once; the per-head 64-column
            # blocks are overwritten by the V projection copies, leaving
            # the denominator ones-columns (64::65) intact
            for j in range(NJ):
                nc.vector.memset(v[j][:], 1.0)
            warm = psp.tile([128, 512], F32, name="warm", tag="pj", bufs=2)

            def dummies(n):
                for _ in range(n):
                    nc.tensor.matmul(warm[:], wsrc[:, 0:128], wsrc[:],
                                     start=True, stop=True)

            # ---- background PE work: projection groups through the 2-bank
            # "pj" psum slots
            def emit_qkt_group(which, p, ncc, tag="pj", pre=False):
                if pre:
                    wf = (lambda k: wqk0s[:, k * 256:k * 256 + 128]) \
                        if which == "q" else \
                        (lambda k: wqk0s[:, k * 256 + 128:k * 256 + 256])
                else:
                    wf = wq if which == "q" else wk
                dst = qt[p] if which == "q" else kt[p]
                ps = psp.tile([128, 512], F32, name="pj", tag=tag, bufs=2)
                for k in range(NK):
                    w_k = wf(k) if pre else wf(k)[:, p * 128:(p + 1) * 128]
                    nc.tensor.matmul(
                        ps[:], w_k, xt_cols(k, ncc * 512, 512),
                        start=(k == 0), stop=(k == NK - 1))
                nc.vector.tensor_copy(
                    out=dst[:, ncc * 512:(ncc + 1) * 512], in_=ps[:])

            def emit_v_group(j, tag="pj"):
                ps = psp.tile([128, 512], F32, name="pj", tag=tag, bufs=2)
                for k in range(NK):
                    nc.tensor.matmul(
                        ps[:], xt_cols(k, j * 128, 128), wv(k),
                        start=(k == 0), stop=(k == NK - 1))
                # per-head rectangular copies into the 65-col V' layout
                for hh_ in range(8):
                    nc.vector.tensor_copy(
                        out=v[j][:, hh_ * 65:hh_ * 65 + 64],
                        in_=ps[:, hh_ * 64:(hh_ + 1) * 64])

            def emit_outproj_qtl(qb, qtl, tail=False):
                ots = [ot_tiles[(qb, p)] for p in range(NP)]
                r0 = qb * 1024 + qtl * 128
                osb = smp.tile([128, DIM], BF16, name="osb", tag="osb", bufs=4)
                if tail:
                    # s- and pj-tag psum banks are free after the last exp;
                    # rotate the dm-half groups through both for a 4-deep
                    # pipeline, copies alternating ACT/DVE
                    for dm in range(2):
                        tg = "s" if (qtl + dm) % 2 == 0 else "pj"
                        ps = psp.tile([128, 512], F32, name="tps", tag=tg,
                                      bufs=2)
                        for p in range(NP):
                            nc.tensor.matmul(
                                ps[:],
                                ots[p][:, qtl * 128:(qtl + 1) * 128],
                                wo(p)[:, dm * 512:(dm + 1) * 512],
                                start=(p == 0), stop=(p == NP - 1))
                        if dm == 0:
                            nc.scalar.copy(
                                out=osb[:, 0:512], in_=ps[:])
                        else:
                            nc.vector.tensor_copy(
                                out=osb[:, 512:1024], in_=ps[:])
                else:
                    for dm in range(2):
                        ps = psp.tile([128, 512], F32, name="pj", tag="pj",
                                      bufs=2)
                        for p in range(NP):
                            nc.tensor.matmul(
                                ps[:], ots[p][:, qtl * 128:(qtl + 1) * 128],
                                wo(p)[:, dm * 512:(dm + 1) * 512],
                                start=(p == 0), stop=(p == NP - 1))
                        nc.vector.tensor_copy(
                            out=osb[:, dm * 512:(dm + 1) * 512], in_=ps[:])
                nc.sync.dma_start(out=out_d[r0:r0 + 128, :], in_=osb[:])

            # ---- attnV backlog: head h's attention-times-V runs during
            # head h+1's S/exp stream, one legal psum group per q-tile.
            ot_tiles = {}
            onp_tiles = {}

            def emit_attnv_qtl(pts, h, qb, qtl, slot):
                o_t = psp.tile([128, 65], F32, name="oq", tag=f"o{slot}",
                               bufs=1)
                for j in range(NJ):
                    nc.tensor.matmul(
                        o_t[:],
                        pts[j][:, qtl * 128:(qtl + 1) * 128],
                        v[j][:, h * 65:h * 65 + 65],
                        start=(j == 0), stop=(j == NJ - 1))
                # normalize: ONP[:, qtl*128 + hh*64 + d] = O[:, d] / O[:, 64]
                p, hh = h // 2, h % 2
                if (p, qb) not in onp_tiles:
                    onp_tiles[(p, qb)] = onp_p.tile(
                        [128, 1024], BF16, name=f"onp{p}", tag=f"onp{p}")
                onp_t = onp_tiles[(p, qb)]
                denr = smp.tile([128, 1], F32, name="denr", tag="denr", bufs=4)
                with nc.allow_low_precision(reason="fp32 recip"):
                    nc.vector.reciprocal(denr[:], o_t[:, 64:65])
                nc.vector.tensor_scalar_mul(
                    onp_t[:, qtl * 128 + hh * 64:qtl * 128 + hh * 64 + 64],
                    o_t[:, 0:64], denr[:, 0:1])

            def finish_pair(p, qb):
                # PE-transpose ONP -> OT_p [128 d, 1024 q] for this q block
                onp_t = onp_tiles[(p, qb)]
                tp = psp.tile([128, 1024], BF16, name="tp", tag="pj", bufs=2)
                for qtl in range(NQT):
                    nc.tensor.transpose(
                        tp[:, qtl * 128:(qtl + 1) * 128],
                        onp_t[:, qtl * 128:(qtl + 1) * 128], ident)
                ot = onp_p.tile([128, 1024], BF16, name=f"ot{p}", tag=f"ot{p}")
                nc.vector.tensor_copy(out=ot[:], in_=tp[:])
                ot_tiles[(qb, p)] = ot

            # background emission schedule: iter t = (qb*8 + h)*16 + j.
            # (deadline_iter, earliest_iter, fn, args)
            groups = []
            for j in range(1, NJ):
                # all V' tiles are read by head-0's backlog from iter 16
                groups.append((j, 0, emit_v_group, (j,)))
            for p in range(NP):
                for c in range(4):  # KT chunk c first read at t=32p+4c
                    if p == 0 and c == 0:
                        continue  # prelude
                    groups.append((32 * p + 4 * c - 3, 0,
                                   emit_qkt_group, ("k", p, c)))
                for ncc in range(4):
                    if p == 0 and ncc in (0, 1):
                        continue  # prelude
                    first = 128 * (ncc // 2) + 32 * p
                    groups.append((first - 4, 0, emit_qkt_group, ("q", p, ncc)))
            for qtl in range(NQT):  # q-block 0 outproj during q-block 1
                groups.append((250, 140 + 2 * qtl, emit_outproj_qtl, (0, qtl)))

            q_bg = deque(sorted(groups, key=lambda g: (g[0], g[1])))
            bg = {}
            last_emit = -10
            for tt in range(NQB * 8 * NJ):
                while q_bg and q_bg[0][0] <= tt:
                    g = q_bg.popleft()
                    bg.setdefault(tt, []).append((g[2], g[3]))
                    last_emit = tt
                if q_bg and tt - last_emit >= 9 and q_bg[0][1] <= tt:
                    g = q_bg.popleft()
                    bg.setdefault(tt, []).append((g[2], g[3]))
                    last_emit = tt

            # ---- prelude: just enough for S(h0, j0)
            dummies(8)
            emit_qkt_group("q", 0, 0, tag="s", pre=True)
            dummies(2)
            emit_qkt_group("q", 0, 1, tag="s", pre=True)
            dummies(2)
            emit_qkt_group("k", 0, 0, tag="pj", pre=True)
            emit_v_group(0)

            # ---- main loop
            cur_pts = []          # P^T tiles of the in-flight head
            backlog = deque()     # (pts, h, qb, qtl) attnV tasks
            slot_ctr = 0
            pend_pair = None

            for t in range(NQB * 8 * NJ):
                qb, r = divmod(t, 8 * NJ)
                h, i = divmod(r, NJ)
                hh, p = h % 2, h // 2
                j = i
                # S^T tile for (qb, h, j): two bank-sized single groups
                s_ps = psp.tile([128, 1024], F32, name="s", tag="s", bufs=2)
                for sc in range(2):
                    nc.tensor.matmul(
                        s_ps[:, sc * 512:(sc + 1) * 512],
                        kt[p][hh * 64:hh * 64 + 64, j * 128:(j + 1) * 128],
                        qt[p][hh * 64:hh * 64 + 64,
                              qb * 1024 + sc * 512:qb * 1024 + (sc + 1) * 512],
                        start=True, stop=True)
                pt_t = ptp.tile([128, 1024], BF16, name="pt", tag="pt")
                nc.scalar.activation(pt_t[:], s_ps[:], AF.Exp, scale=0.125)
                cur_pts.append(pt_t)
                # drain up to 3 backlog attnV groups (prev head's); drain
                # BEFORE the fill so a head's groups start strictly after
                # its last iteration (all V'/P^T writers already emitted)
                for _ in range(2):
                    if not backlog:
                        break
                    pts_, bh, bqb, bqtl = backlog.popleft()
                    emit_attnv_qtl(pts_, bh, bqb, bqtl, slot_ctr % 2)
                    slot_ctr += 1
                    if bqtl == NQT - 1 and bh % 2 == 1:
                        pend_pair = (bh // 2, bqb)
                if pend_pair is not None and not backlog and i >= 5:
                    # deferred so the DVE norm chain has fully drained and
                    # the transposes never stall the in-order PE queue
                    finish_pair(*pend_pair)
                    pend_pair = None
                if i == NJ - 1:
                    # head finished streaming: queue its attnV backlog
                    for qtl in range(NQT):
                        backlog.append((cur_pts, h, qb, qtl))
                    cur_pts = []
                # background projection / output-projection groups
                for fn, a in bg.get(t, ()):
                    fn(*a)

            # drain: last head's backlog, its pair, final outproj
            while backlog:
                pts_, bh, bqb, bqtl = backlog.popleft()
                emit_attnv_qtl(pts_, bh, bqb, bqtl, slot_ctr % 2)
                slot_ctr += 1
            finish_pair(3, NQB - 1)
            for qtl in range(NQT):
                emit_outproj_qtl(NQB - 1, qtl, tail=True)
    nc.finalize()
    return nc


def kernel(x, Wq, Wk, Wv, Wo, bo, _trace=False):
    npdt = mybir.dt.np(BF16)
    x = np.asarray(x, np.float32)
    bo = np.asarray(bo, np.float32)
    Wq, Wk, Wv = (np.asarray(a, np.float32) for a in (Wq, Wk, Wv))
    Wo = np.asarray(Wo, np.float32)

    if "nc" not in _CACHE:
        _CACHE["nc"] = build()
    nc = _CACHE["nc"]

    idon_in = np.concatenate(
        [np.eye(128, dtype=np.float32),
         np.ones((128, 1), np.float32)], axis=1).astype(npdt)
    in_maps = []
    for c in range(8):
        b, hb = c // 2, c % 2
        sl = slice(hb * HL, (hb + 1) * HL)
        xT = x[b].T  # [1024, 2048]
        xA = xT[:, 0:1024].reshape(NK, 128, 1024).transpose(1, 0, 2)
        xB = xT[:, 1024:2048].reshape(NK, 128, 1024).transpose(1, 0, 2)
        xtp = np.concatenate([xA.reshape(128, NK * 1024),
                              xB.reshape(128, NK * 1024)], axis=1)
        wqk = np.concatenate(
            [Wq[:, sl].reshape(NK, 128, HL),
             Wk[:, sl].reshape(NK, 128, HL)],
            axis=2).transpose(1, 0, 2).reshape(128, NK * 2 * HL)
        wqk0 = np.concatenate(
            [Wq[:, sl][:, 0:128].reshape(NK, 128, 128),
             Wk[:, sl][:, 0:128].reshape(NK, 128, 128)],
            axis=2).transpose(1, 0, 2).reshape(128, NK * 256)
        wvp = Wv[:, sl].reshape(NK, 128, HL).transpose(1, 0, 2).reshape(
            128, NK * HL)
        wop = Wo[sl, :].reshape(NP, 128, DIM).transpose(1, 0, 2).reshape(
            128, NP * DIM)
        in_maps.append({
            "xtp": np.ascontiguousarray(xtp).astype(npdt),
            "wqk": np.ascontiguousarray(wqk).astype(npdt),
            "wqk0": np.ascontiguousarray(wqk0).astype(npdt),
            "wvp": np.ascontiguousarray(wvp).astype(npdt),
            "wop": np.ascontiguousarray(wop).astype(npdt),
            "idon": idon_in,
        })
    res = run_bass_kernel_spmd(nc, in_maps, list(range(8)), trace=_trace)
    out = np.empty((4, N, DIM), np.float32)
    for b in range(4):
        out[b] = (res.results[2 * b]["out"].astype(np.float32)
                  + res.results[2 * b + 1]["out"].astype(np.float32) + bo)
    if _trace:
        return out, res
    return out
